# revision 1
# baseline (speedup 1.0000x reference)
"""Trainium2 Bass kernel for CustomStellarModel2 (GNN message passing).

Self-contained: host-side sharding/preprocessing + Bass/Tile kernel
compiled and run on 8 NeuronCores via PJRT (axon), then unsharded.

Strategy:
  - Nodes sharded contiguously across 8 cores (12500/core, padded to 12544).
  - Edges partitioned by dst owner, grouped by dst-block of 128 nodes,
    sorted by src, split into 4 banks of <=32768 rows so src indices fit
    int16 for dma_gather.
  - Per-edge gathers of src-node features (xl in layer 1, q|v in layer 2)
    via dma_gather on 4 SWDGE queues.
  - Dst-side per-edge expansion (gamma/beta, k) and segment reduction done
    with host-precomputed one-hot matrices (fp8, exact) on TensorE; the
    segment-sum accumulates in PSUM. No scatter needed.
  - Two AllGather collectives replicate the gather tables (xl, qv).
"""
import math
import numpy as np
import ml_dtypes

BF16 = ml_dtypes.bfloat16
FP8 = ml_dtypes.float8_e4m3

# Problem constants (hardcoded per contest rules); _config() allows
# small-scale overrides for simulator testing.
IN_DIM, H, C_OUT = 64, 128, 20
N_CORES = 8


def _config(n=100000, e=1600000, bank=32768, gcap=1024):
    global N, E, NC, G, NP, NFULL, BANK, N_BANKS, GCAP
    N, E, BANK, GCAP = n, e, bank, gcap
    NC = N // N_CORES            # real nodes per core
    G = math.ceil(NC / 128)      # groups per core
    NP = G * 128                 # padded nodes per core
    NFULL = NP * N_CORES         # padded table rows
    N_BANKS = math.ceil(NFULL / BANK)


_config()
ABLATE = set()


# ---------------------------------------------------------------------------
# Host-side preprocessing
# ---------------------------------------------------------------------------

def _prep_edges(edge_index):
    """Partition/sort/pad edges; build common (cross-core) schedule,
    per-core idx blobs, Gt/GtT one-hot blobs, and inv_cnt."""
    src = edge_index[0].astype(np.int64)
    dst = edge_index[1].astype(np.int64)
    core_of = dst // NC
    per_core = []
    for c in range(N_CORES):
        m = core_of == c
        s, d = src[m], dst[m] - c * NC
        # padded global row id of source node
        sc = s // NC
        s_pad = sc * NP + (s - sc * NC)
        g = d // 128
        bank = s_pad // BANK
        order = np.lexsort((s_pad, bank, g))
        per_core.append((s_pad[order], d[order], g[order], bank[order]))

    # counts per (core, group, bank)
    cnts = np.zeros((N_CORES, G, N_BANKS), np.int64)
    for c in range(N_CORES):
        _, _, g, b = per_core[c]
        np.add.at(cnts, (c, g, b), 1)
    # common schedule: pad to max over cores, round to 128, split into <=GCAP
    npad = ((cnts.max(axis=0) + 127) // 128) * 128  # [G, N_BANKS]
    # per-(g,b) gather instructions: list of idx counts (multiples of 128)
    sched = []  # per group: list of (bank, n_idx, col_off)
    Cg = np.zeros(G, np.int64)
    for g in range(G):
        items = []
        col = 0
        for b in range(N_BANKS):
            n = int(npad[g, b])
            while n > 0:
                take = min(n, GCAP)
                items.append((b, take, col))
                col += take // 128
                n -= take
        sched.append(items)
        Cg[g] = col
    TOTC = int(Cg.sum())
    g_coff = np.zeros(G + 1, np.int64)
    g_coff[1:] = np.cumsum(Cg)

    # per-core blobs
    idx_blobs, gt_blobs, gtt_blobs = [], [], []
    inv_cnts = []
    for c in range(N_CORES):
        s_pad, d, g, b = per_core[c]
        # in-degree of real nodes (for mean)
        cnt = np.zeros(NP, np.float32)
        np.add.at(cnt, d, 1.0)
        inv = 1.0 / np.maximum(cnt, 1.0)
        inv_cnts.append(inv.reshape(G, 128).T.copy())  # [128, G]

        # slot assignment: for each (g, bank), edges fill columns in order
        idx16 = np.zeros((128, TOTC * 128 // 16), np.int16)
        gt = np.zeros((128, TOTC * 128), FP8)
        gtt = np.zeros((128, TOTC * 128), FP8)
        # build per-(g,b) runs
        # edges already sorted by (g, bank, src)
        run_starts = np.zeros((G, N_BANKS), np.int64)
        np.cumsum(cnts[c].reshape(-1))
        flat = np.concatenate([[0], np.cumsum(cnts[c].reshape(-1))])
        run_starts = flat[:-1].reshape(G, N_BANKS)
        for gi in range(G):
            colbase = g_coff[gi]
            boff = 0
            for bi in range(N_BANKS):
                n_real = int(cnts[c, gi, bi])
                n_p = int(npad[gi, bi])
                e0 = int(run_starts[gi, bi])
                rel = np.zeros(n_p, np.int64)  # bank-relative src idx
                drel = np.full(n_p, 128, np.int64)  # 128 => padding
                rel[:n_real] = s_pad[e0:e0 + n_real] - bi * BANK
                drel[:n_real] = d[e0:e0 + n_real] - gi * 128
                # slot i -> (p=i%128, col=colbase+boff+i//128)
                i = np.arange(n_p)
                p = i % 128
                col = colbase + boff + i // 128
                # idx blob: idx i within one gather instr at [i%16, w0+i//16]
                # instructions of GCAP idxs starting at boff
                j = 0
                while j < n_p:
                    take = min(n_p - j, GCAP)
                    w0 = (colbase + boff) * 8 + j // 16
                    ii = np.arange(take)
                    blk = np.zeros((16, (take + 15) // 16), np.int16)
                    blk[ii % 16, ii // 16] = rel[j:j + take].astype(np.int16)
                    for rep in range(8):
                        idx16[rep * 16:(rep + 1) * 16, w0:w0 + take // 16] = blk
                    j += take
                # one-hot fills (skip padding slots)
                mreal = drel < 128
                pp, cc, ss = p[mreal], col[mreal], drel[mreal]
                gt[pp, cc * 128 + ss] = 1.0
                gtt[ss, cc * 128 + pp] = 1.0
                boff += n_p // 128
        idx_blobs.append(idx16)
        gt_blobs.append(gt)
        gtt_blobs.append(gtt)

    meta = {
        "sched": sched, "Cg": Cg.astype(int), "g_coff": g_coff, "TOTC": TOTC,
    }
    return meta, idx_blobs, gt_blobs, gtt_blobs, inv_cnts


def _prep_weights(inp):
    f = lambda a: np.ascontiguousarray(a, dtype=np.float32)
    W1T = f(inp["W1"]).T.astype(BF16)                      # [64,128]
    # reference: beta = bg[:, :H], gamma = bg[:, H:].  The kernel consumes
    # [gamma | beta] column order, so swap halves here.
    WfT = f(inp["Wf"]).T
    WfT_gb = np.concatenate([WfT[:, H:], WfT[:, :H]], axis=1)
    bf_gb = np.concatenate([f(inp["bf"])[H:], f(inp["bf"])[:H]])
    WfsT = f(inp["Wfs"]).T
    WfsT_gb = np.concatenate([WfsT[:, H:], WfsT[:, :H]], axis=1)
    bfs_gb = np.concatenate([f(inp["bfs"])[H:], f(inp["bfs"])[:H]])
    R1 = np.concatenate(
        [f(inp["Wl"]).T, WfT_gb, WfsT_gb, f(inp["Wls"]).T],
        axis=1).astype(BF16)                               # [128, 768]
    bias1 = np.concatenate(
        [np.zeros(H, np.float32), bf_gb, bfs_gb,
         np.zeros(H, np.float32)])[None, :].astype(BF16)   # [1, 768]
    R2 = np.concatenate(
        [f(inp["Wk"]).T, f(inp["Wskip"]).T, f(inp["Wq"]).T, f(inp["Wv"]).T],
        axis=1).astype(BF16)                               # [128, 512]
    bias2 = np.concatenate(
        [f(inp["bk"]), f(inp["bres"]), f(inp["bq"]), f(inp["bv"])]
    )[None, :].astype(BF16)                                # [1, 512]
    WfcT = f(inp["Wfc"]).T.astype(BF16)                    # [128, 20]
    bfc = f(inp["bfc"])[None, :].astype(BF16)              # [1, 20]
    b1 = f(inp["b1"])[None, :].astype(BF16)                # [1, 128]
    return dict(W1T=W1T, R1=R1, bias1=bias1, R2=R2, bias2=bias2,
                WfcT=WfcT, bfc=bfc, b1=b1)


# ---------------------------------------------------------------------------
# Bass kernel builder
# ---------------------------------------------------------------------------

def _build(meta):
    import concourse.bass as bass
    import concourse.bacc as bacc
    import concourse.mybir as mybir
    import concourse.tile as tile
    from concourse import library_config
    from concourse.masks import make_identity

    dt = mybir.dt
    sched, Cg, g_coff, TOTC = (meta["sched"], meta["Cg"], meta["g_coff"],
                               meta["TOTC"])
    CMAX = int(max(Cg))

    nc = bacc.Bacc("TRN2", target_bir_lowering=False, debug=False,
                   num_devices=N_CORES, dynamic_dma_scratch_size=131072,
                   num_swdge_queues=4)

    # ---- external inputs ----
    xT = nc.dram_tensor("xT", [IN_DIM, NP], dt.bfloat16,
                        kind="ExternalInput").ap()
    W1T = nc.dram_tensor("W1T", [IN_DIM, H], dt.bfloat16,
                         kind="ExternalInput").ap()
    R1 = nc.dram_tensor("R1", [H, 6 * H], dt.bfloat16,
                        kind="ExternalInput").ap()
    bias1 = nc.dram_tensor("bias1", [1, 6 * H], dt.bfloat16,
                           kind="ExternalInput").ap()
    R2 = nc.dram_tensor("R2", [H, 4 * H], dt.bfloat16,
                        kind="ExternalInput").ap()
    bias2 = nc.dram_tensor("bias2", [1, 4 * H], dt.bfloat16,
                           kind="ExternalInput").ap()
    WfcT = nc.dram_tensor("WfcT", [H, C_OUT], dt.bfloat16,
                          kind="ExternalInput").ap()
    bfc = nc.dram_tensor("bfc", [1, C_OUT], dt.bfloat16,
                         kind="ExternalInput").ap()
    b1 = nc.dram_tensor("b1", [1, H], dt.bfloat16, kind="ExternalInput").ap()
    idxb = nc.dram_tensor("idxb", [128, TOTC * 8], dt.int16,
                          kind="ExternalInput").ap()
    gtb = nc.dram_tensor("gtb", [128, TOTC * 128], dt.float8e4,
                         kind="ExternalInput").ap()
    gttb = nc.dram_tensor("gttb", [128, TOTC * 128], dt.float8e4,
                          kind="ExternalInput").ap()
    invc = nc.dram_tensor("invc", [128, G], dt.float32,
                          kind="ExternalInput").ap()
    outT = nc.dram_tensor("outT", [C_OUT, NP], dt.float32,
                          kind="ExternalOutput").ap()

    with tile.TileContext(nc) as tc:
        with (
            tc.tile_pool(name="dram", bufs=1, space="DRAM") as dp,
            tc.tile_pool(name="const", bufs=1) as cp,
        ):
            nc.gpsimd.load_library(library_config.mlp)
            # DRAM intermediates
            xl_loc = dp.tile([NP, H], dt.bfloat16)
            gb_loc = dp.tile([NP, 2 * H], dt.bfloat16)
            skip_loc = dp.tile([NP, H], dt.bfloat16)
            x2T_loc = dp.tile([G, 128, H], dt.bfloat16)
            k_loc = dp.tile([NP, H], dt.bfloat16)
            skip2_loc = dp.tile([NP, H], dt.bfloat16)
            qv_loc = dp.tile([NP, 2 * H], dt.bfloat16)
            xl_full = dp.tile([NFULL, H], dt.bfloat16)
            qv_full = dp.tile([NFULL, 2 * H], dt.bfloat16)

            # constants in SBUF
            ones_col = cp.tile([1, 512], dt.bfloat16)
            nc.vector.memset(ones_col[:], 1.0)
            ident = cp.tile([128, 128], dt.bfloat16)
            make_identity(nc, ident[:])
            w1t_t = cp.tile([IN_DIM, H], dt.bfloat16)
            nc.sync.dma_start(out=w1t_t[:], in_=W1T[:])
            r1_t = cp.tile([H, 6 * H], dt.bfloat16)
            nc.sync.dma_start(out=r1_t[:], in_=R1[:])
            bias1_t = cp.tile([1, 6 * H], dt.bfloat16)
            nc.sync.dma_start(out=bias1_t[:], in_=bias1[:])
            r2_t = cp.tile([H, 4 * H], dt.bfloat16)
            nc.sync.dma_start(out=r2_t[:], in_=R2[:])
            bias2_t = cp.tile([1, 4 * H], dt.bfloat16)
            nc.sync.dma_start(out=bias2_t[:], in_=bias2[:])
            wfc_t = cp.tile([H, C_OUT], dt.bfloat16)
            nc.sync.dma_start(out=wfc_t[:], in_=WfcT[:])
            bfc_t = cp.tile([1, C_OUT], dt.bfloat16)
            nc.sync.dma_start(out=bfc_t[:], in_=bfc[:])
            b1_t = cp.tile([1, H], dt.bfloat16)
            nc.sync.dma_start(out=b1_t[:], in_=b1[:])
            invc_t = cp.tile([128, G], dt.float32)
            nc.sync.dma_start(out=invc_t[:], in_=invc[:])

            # ---------------- P1: dense layer 1 (local nodes) -------------
            with (
                tc.tile_pool(name="p1sb", bufs=3) as sb,
                tc.tile_pool(name="p1ps", bufs=2, space="PSUM") as ps,
                tc.tile_pool(name="p1ps2", bufs=2, space="PSUM") as ps2,
            ):
                NB = 512
                for s0 in range(0, NP, NB):
                    n = min(NB, NP - s0)
                    xt_t = sb.tile([IN_DIM, n], dt.bfloat16, tag="xt")
                    nc.sync.dma_start(out=xt_t[:], in_=xT[:, s0:s0 + n])
                    p1 = ps.tile([128, NB], dt.float32, tag="p1")
                    nc.tensor.matmul(p1[:, :n], lhsT=w1t_t[:], rhs=xt_t[:],
                                     start=True, stop=False)
                    nc.tensor.matmul(p1[:, :n], lhsT=b1_t[:],
                                     rhs=ones_col[:, :n],
                                     start=False, stop=True)
                    x1t = sb.tile([128, NB], dt.bfloat16, tag="x1t")
                    nc.scalar.activation(x1t[:, :n], p1[:, :n],
                                         mybir.ActivationFunctionType.Relu)
                    for nb in range(0, n, 128):
                        m = min(128, n - nb)
                        p2 = ps2.tile([128, 6 * H], dt.float32, tag="p2")
                        lhsT = x1t[:, nb:nb + m]
                        nc.tensor.matmul(p2[:m, :512], lhsT=lhsT,
                                         rhs=r1_t[:, :512],
                                         start=True, stop=False)
                        nc.tensor.matmul(p2[:m, :512], lhsT=ones_col[:, :m],
                                         rhs=bias1_t[:, :512],
                                         start=False, stop=True)
                        nc.tensor.matmul(p2[:m, 512:], lhsT=lhsT,
                                         rhs=r1_t[:, 512:],
                                         start=True, stop=False)
                        nc.tensor.matmul(p2[:m, 512:], lhsT=ones_col[:, :m],
                                         rhs=bias1_t[:, 512:],
                                         start=False, stop=True)
                        row0 = s0 + nb
                        # xl and gamma-beta out
                        xlgb = sb.tile([128, 3 * H], dt.bfloat16, tag="xlgb")
                        nc.scalar.copy(xlgb[:m, :], p2[:m, :3 * H])
                        nc.sync.dma_start(out=xl_loc[row0:row0 + m, :],
                                          in_=xlgb[:m, :H])
                        nc.sync.dma_start(out=gb_loc[row0:row0 + m, :],
                                          in_=xlgb[:m, H:])
                        # FiLM skip: relu(gs * xls + bs)
                        # (copy PSUM slice to SBUF first: DVE tensor_tensor
                        # may read at most one PSUM operand)
                        sks = sb.tile([128, 3 * H], dt.bfloat16, tag="sks")
                        nc.scalar.copy(sks[:m, :], p2[:m, 384:768])
                        tmp = sb.tile([128, H], dt.bfloat16, tag="tmp")
                        nc.vector.tensor_mul(tmp[:m, :], sks[:m, :H],
                                             sks[:m, 2 * H:])
                        pre = sb.tile([128, H], dt.bfloat16, tag="pre")
                        nc.vector.tensor_add(pre[:m, :], tmp[:m, :],
                                             sks[:m, H:2 * H])
                        sk = sb.tile([128, H], dt.bfloat16, tag="sk")
                        nc.vector.tensor_scalar_max(sk[:m, :], pre[:m, :], 0.0)
                        nc.sync.dma_start(out=skip_loc[row0:row0 + m, :],
                                          in_=sk[:m, :])

            # ---------------- P2: AllGather xl ----------------------------
            nc.gpsimd.collective_compute(
                "AllGather", mybir.AluOpType.bypass,
                replica_groups=[list(range(N_CORES))],
                ins=[xl_loc[:]], outs=[xl_full[:]],
            )

            # ---------------- P3: FiLM edge phase -------------------------
            with (
                tc.tile_pool(name="p3sb", bufs=2) as sb,
                tc.tile_pool(name="p3g", bufs=2) as gpool,
                tc.tile_pool(name="p3ps", bufs=2, space="PSUM") as pse,
                tc.tile_pool(name="p3pa", bufs=2, space="PSUM") as psa,
                tc.tile_pool(name="p3pt", bufs=2, space="PSUM") as pst,
            ):
                for g in range(G):
                    C = int(Cg[g])
                    co = int(g_coff[g])
                    gb_g = sb.tile([128, 2 * H], dt.bfloat16, tag="gbg")
                    nc.sync.dma_start(out=gb_g[:],
                                      in_=gb_loc[g * 128:(g + 1) * 128, :])
                    gt_g = sb.tile([128, CMAX * 128], dt.float8e4, tag="gt")
                    nc.sync.dma_start(out=gt_g[:, :C * 128],
                                      in_=gtb[:, co * 128:(co + C) * 128])
                    gtt_g = sb.tile([128, CMAX * 128], dt.float8e4, tag="gtt")
                    nc.sync.dma_start(out=gtt_g[:, :C * 128],
                                      in_=gttb[:, co * 128:(co + C) * 128])
                    idx_g = sb.tile([128, CMAX * 8], dt.int16, tag="idx")
                    nc.sync.dma_start(out=idx_g[:, :C * 8],
                                      in_=idxb[:, co * 8:(co + C) * 8])
                    skp_g = sb.tile([128, H], dt.bfloat16, tag="skp")
                    nc.sync.dma_start(out=skp_g[:],
                                      in_=skip_loc[g * 128:(g + 1) * 128, :])
                    # gathers
                    xg = gpool.tile([128, CMAX, H], dt.bfloat16, tag="xg")
                    if "gather" in ABLATE:
                        nc.vector.memset(xg[:], 0.0)
                    for qi, (b, nidx, coloff) in enumerate(
                            [] if "gather" in ABLATE else sched[g]):
                        nrow = min(BANK, NFULL - b * BANK)
                        nc.gpsimd.dma_gather(
                            xg[:, coloff:coloff + nidx // 128, :],
                            xl_full[b * BANK:b * BANK + nrow, :],
                            idx_g[:, coloff * 8:coloff * 8 + nidx // 16],
                            nidx, nidx, H, queue_num=qi % 4,
                        )
                    # expansion of gamma/beta to edges (4-chunk batches)
                    gbe = gpool.tile([128, CMAX, 2 * H], dt.bfloat16,
                                     tag="gbe")
                    if "expand" in ABLATE:
                        nc.vector.memset(gbe[:], 0.0)
                    for c0 in ([] if "expand" in ABLATE else
                               range(0, C, 4)):
                        cn = min(4, C - c0)
                        pe = pse.tile([128, 4, 2 * H], dt.float32, tag="pe")
                        for j in range(cn):
                            c = c0 + j
                            nc.tensor.matmul(
                                pe[:, j, :],
                                lhsT=gtt_g[:, c * 128:(c + 1) * 128],
                                rhs=gb_g[:], start=True, stop=True)
                        nc.scalar.copy(gbe[:, c0:c0 + cn, :], pe[:, :cn, :])
                    # msg = relu(ge * xg + be)
                    msg = gpool.tile([128, CMAX, H], dt.bfloat16, tag="msg")
                    if "elem" in ABLATE:
                        nc.vector.memset(msg[:], 0.0)
                    if "elem" not in ABLATE:
                        m0 = gpool.tile([128, CMAX, H], dt.bfloat16, tag="m0")
                        m1 = gpool.tile([128, CMAX, H], dt.bfloat16, tag="m1")
                        nc.vector.tensor_mul(m0[:, :C, :], gbe[:, :C, :H],
                                             xg[:, :C, :])
                        nc.vector.tensor_add(m1[:, :C, :], m0[:, :C, :],
                                             gbe[:, :C, H:])
                        nc.vector.tensor_scalar_max(msg[:, :C, :],
                                                    m1[:, :C, :], 0.0)
                    # segment reduce via Gt matmuls
                    pa = psa.tile([128, H], dt.float32, tag="pa")
                    for c in (range(1) if "reduce" in ABLATE else range(C)):
                        nc.tensor.matmul(pa[:],
                                         lhsT=gt_g[:, c * 128:(c + 1) * 128],
                                         rhs=msg[:, c, :],
                                         start=(c == 0),
                                         stop=(c == C - 1
                                               or "reduce" in ABLATE))
                    # x2 = relu(skip + agg/cnt)
                    aggm = sb.tile([128, H], dt.float32, tag="aggm")
                    nc.vector.tensor_scalar_mul(aggm[:], pa[:],
                                                invc_t[:, g:g + 1])
                    x2p = sb.tile([128, H], dt.float32, tag="x2p")
                    nc.vector.tensor_add(x2p[:], aggm[:], skp_g[:])
                    x2 = sb.tile([128, H], dt.bfloat16, tag="x2")
                    nc.vector.tensor_scalar_max(x2[:], x2p[:], 0.0)
                    # x2 transpose for dense-2 lhsT
                    pt = pst.tile([128, H], dt.bfloat16, tag="pt")
                    nc.tensor.transpose(pt[:], x2[:], ident[:])
                    x2t = sb.tile([128, H], dt.bfloat16, tag="x2t")
                    nc.scalar.copy(x2t[:], pt[:])
                    nc.sync.dma_start(out=x2T_loc[g, :, :], in_=x2t[:])

            # ---------------- P4: dense layer 2 (local) -------------------
            with (
                tc.tile_pool(name="p4sb", bufs=3) as sb,
                tc.tile_pool(name="p4ps", bufs=2, space="PSUM") as ps,
            ):
                for g in range(G):
                    x2t = sb.tile([128, H], dt.bfloat16, tag="x2t")
                    nc.sync.dma_start(out=x2t[:], in_=x2T_loc[g, :, :])
                    p2 = ps.tile([128, 4 * H], dt.float32, tag="p2")
                    nc.tensor.matmul(p2[:], lhsT=x2t[:], rhs=r2_t[:],
                                     start=True, stop=False)
                    nc.tensor.matmul(p2[:], lhsT=ones_col[:, :128],
                                     rhs=bias2_t[:], start=False, stop=True)
                    kk = sb.tile([128, 2 * H], dt.bfloat16, tag="kk")
                    nc.scalar.copy(kk[:], p2[:, :2 * H])
                    row0 = g * 128
                    nc.sync.dma_start(out=k_loc[row0:row0 + 128, :],
                                      in_=kk[:, :H])
                    nc.sync.dma_start(out=skip2_loc[row0:row0 + 128, :],
                                      in_=kk[:, H:])
                    qv = sb.tile([128, 2 * H], dt.bfloat16, tag="qv")
                    nc.scalar.copy(qv[:], p2[:, 2 * H:])
                    nc.sync.dma_start(out=qv_loc[row0:row0 + 128, :],
                                      in_=qv[:])

            # ---------------- P5: AllGather qv ----------------------------
            nc.gpsimd.collective_compute(
                "AllGather", mybir.AluOpType.bypass,
                replica_groups=[list(range(N_CORES))],
                ins=[qv_loc[:]], outs=[qv_full[:]],
            )

            # ---------------- P6: ResGated edge phase ---------------------
            with (
                tc.tile_pool(name="p6sb", bufs=2) as sb,
                tc.tile_pool(name="p6g", bufs=2) as gpool,
                tc.tile_pool(name="p6ps", bufs=2, space="PSUM") as psk,
                tc.tile_pool(name="p6pa", bufs=2, space="PSUM") as psa,
                tc.tile_pool(name="p6pt", bufs=2, space="PSUM") as pst,
                tc.tile_pool(name="p6pl", bufs=2, space="PSUM") as psl,
            ):
                for g in range(G):
                    C = int(Cg[g])
                    co = int(g_coff[g])
                    k_g = sb.tile([128, H], dt.bfloat16, tag="kg")
                    nc.sync.dma_start(out=k_g[:],
                                      in_=k_loc[g * 128:(g + 1) * 128, :])
                    sk2_g = sb.tile([128, H], dt.bfloat16, tag="sk2")
                    nc.sync.dma_start(out=sk2_g[:],
                                      in_=skip2_loc[g * 128:(g + 1) * 128, :])
                    gt_g = sb.tile([128, CMAX * 128], dt.float8e4, tag="gt6")
                    nc.sync.dma_start(out=gt_g[:, :C * 128],
                                      in_=gtb[:, co * 128:(co + C) * 128])
                    gtt_g = sb.tile([128, CMAX * 128], dt.float8e4,
                                    tag="gtt6")
                    nc.sync.dma_start(out=gtt_g[:, :C * 128],
                                      in_=gttb[:, co * 128:(co + C) * 128])
                    idx_g = sb.tile([128, CMAX * 8], dt.int16, tag="idx6")
                    nc.sync.dma_start(out=idx_g[:, :C * 8],
                                      in_=idxb[:, co * 8:(co + C) * 8])
                    qvg = gpool.tile([128, CMAX, 2 * H], dt.bfloat16,
                                     tag="qvg")
                    if "gather" in ABLATE:
                        nc.vector.memset(qvg[:], 0.0)
                    for qi, (b, nidx, coloff) in enumerate(
                            [] if "gather" in ABLATE else sched[g]):
                        nrow = min(BANK, NFULL - b * BANK)
                        nc.gpsimd.dma_gather(
                            qvg[:, coloff:coloff + nidx // 128, :],
                            qv_full[b * BANK:b * BANK + nrow, :],
                            idx_g[:, coloff * 8:coloff * 8 + nidx // 16],
                            nidx, nidx, 2 * H, queue_num=qi % 4,
                        )
                    # expand k to edges
                    ke = gpool.tile([128, CMAX, H], dt.bfloat16, tag="ke")
                    if "expand" in ABLATE:
                        nc.vector.memset(ke[:], 0.0)
                    for c0 in ([] if "expand" in ABLATE else
                               range(0, C, 4)):
                        cn = min(4, C - c0)
                        pk = psk.tile([128, 4, H], dt.float32, tag="pk")
                        for j in range(cn):
                            c = c0 + j
                            nc.tensor.matmul(
                                pk[:, j, :],
                                lhsT=gtt_g[:, c * 128:(c + 1) * 128],
                                rhs=k_g[:], start=True, stop=True)
                        nc.scalar.copy(ke[:, c0:c0 + cn, :], pk[:, :cn, :])
                    # eta = sigmoid(ke + qg); contrib = eta * vg
                    ctb = gpool.tile([128, CMAX, H], dt.bfloat16, tag="ctb")
                    if "elem" in ABLATE:
                        nc.vector.memset(ctb[:], 0.0)
                    if "elem" not in ABLATE:
                        kq = gpool.tile([128, CMAX, H], dt.bfloat16, tag="kq")
                        eta = gpool.tile([128, CMAX, H], dt.bfloat16,
                                         tag="eta")
                        nc.vector.tensor_add(kq[:, :C, :], ke[:, :C, :],
                                             qvg[:, :C, :H])
                        nc.scalar.activation(
                            eta[:, :C, :], kq[:, :C, :],
                            mybir.ActivationFunctionType.Sigmoid)
                        nc.vector.tensor_mul(ctb[:, :C, :], eta[:, :C, :],
                                             qvg[:, :C, H:])
                    pa = psa.tile([128, H], dt.float32, tag="pa6")
                    for c in (range(1) if "reduce" in ABLATE else range(C)):
                        nc.tensor.matmul(pa[:],
                                         lhsT=gt_g[:, c * 128:(c + 1) * 128],
                                         rhs=ctb[:, c, :],
                                         start=(c == 0),
                                         stop=(c == C - 1
                                               or "reduce" in ABLATE))
                    # x3 = relu(s2 + skip2)
                    x3p = sb.tile([128, H], dt.float32, tag="x3p")
                    nc.vector.tensor_add(x3p[:], pa[:], sk2_g[:])
                    x3 = sb.tile([128, H], dt.bfloat16, tag="x3")
                    nc.vector.tensor_scalar_max(x3[:], x3p[:], 0.0)
                    pt = pst.tile([128, H], dt.bfloat16, tag="pt6")
                    nc.tensor.transpose(pt[:], x3[:], ident[:])
                    x3t = sb.tile([128, H], dt.bfloat16, tag="x3t")
                    nc.scalar.copy(x3t[:], pt[:])
                    # logits_T = Wfc @ x3T + bfc
                    pl = psl.tile([C_OUT, 128], dt.float32, tag="pl")
                    nc.tensor.matmul(pl[:], lhsT=wfc_t[:], rhs=x3t[:],
                                     start=True, stop=False)
                    nc.tensor.matmul(pl[:], lhsT=bfc_t[:],
                                     rhs=ones_col[:, :128],
                                     start=False, stop=True)
                    lt = sb.tile([C_OUT, 128], dt.float32, tag="lt")
                    nc.scalar.copy(lt[:], pl[:])
                    nc.sync.dma_start(out=outT[:, g * 128:(g + 1) * 128],
                                      in_=lt[:])

    nc.compile()
    return nc


# ---------------------------------------------------------------------------
# Runner (PJRT shard_map, compile once)
# ---------------------------------------------------------------------------

class _Runner:
    def __init__(self, nc):
        import jax
        import concourse.mybir as mybir
        from concourse import bass2jax
        from concourse.bass2jax import _bass_exec_p, install_neuronx_cc_hook
        from jax.sharding import Mesh, PartitionSpec
        try:
            from jax.experimental.shard_map import shard_map
        except ImportError:
            from jax.sharding import shard_map
        install_neuronx_cc_hook()
        self.jax = jax
        partition_name = (nc.partition_id_tensor.name
                          if nc.partition_id_tensor else None)
        in_names, out_names, out_avals, zero_outs = [], [], [], []
        for alloc in nc.m.functions[0].allocations:
            if not isinstance(alloc, mybir.MemoryLocationSet):
                continue
            name = alloc.memorylocations[0].name
            if alloc.kind == "ExternalInput":
                if name != partition_name:
                    in_names.append(name)
            elif alloc.kind == "ExternalOutput":
                out_names.append(name)
                shape = tuple(alloc.tensor_shape)
                dtype = mybir.dt.np(alloc.dtype)
                out_avals.append(jax.core.ShapedArray(shape, dtype))
                zero_outs.append(np.zeros(shape, dtype))
        self.in_names, self.out_names = in_names, out_names
        self.out_avals, self.zero_outs = out_avals, zero_outs
        n_params, n_outs = len(in_names), len(out_avals)
        all_in = list(in_names) + list(out_names)
        if partition_name is not None:
            all_in.append(partition_name)

        def _body(*args):
            operands = list(args)
            if partition_name is not None:
                operands.append(bass2jax.partition_id_tensor())
            return tuple(_bass_exec_p.bind(
                *operands, out_avals=tuple(out_avals),
                in_names=tuple(all_in), out_names=tuple(out_names),
                lowering_input_output_aliases=(),
                sim_require_finite=True, sim_require_nnan=True, nc=nc))

        devices = jax.devices()[:N_CORES]
        self.mesh = Mesh(np.asarray(devices), ("core",))
        specs_in = (PartitionSpec("core"),) * (n_params + n_outs)
        specs_out = (PartitionSpec("core"),) * len(out_names)
        self.fn = jax.jit(
            shard_map(_body, mesh=self.mesh, in_specs=specs_in,
                      out_specs=specs_out, check_rep=False),
            keep_unused=True)

    def run(self, in_maps):
        jax = self.jax
        from jax.sharding import NamedSharding, PartitionSpec
        per_core = [[np.asarray(m[n]) for n in self.in_names]
                    for m in in_maps]
        concat = [np.concatenate([per_core[c][i] for c in range(N_CORES)], 0)
                  for i in range(len(self.in_names))]
        zeros = [np.zeros((N_CORES * z.shape[0], *z.shape[1:]), z.dtype)
                 for z in self.zero_outs]
        sh = NamedSharding(self.mesh, PartitionSpec("core"))
        args = [jax.device_put(a, sh) for a in concat + zeros]
        outs = self.fn(*args)
        jax.block_until_ready(outs)
        return [
            {n: np.asarray(outs[i]).reshape(N_CORES,
                                            *self.out_avals[i].shape)[c]
             for i, n in enumerate(self.out_names)}
            for c in range(N_CORES)
        ], (args, outs)


_CACHE = {}


def kernel(**inputs) -> np.ndarray:
    edge_index = np.asarray(inputs["edge_index"])
    x = np.asarray(inputs["x"], dtype=np.float32)

    meta, idx_blobs, gt_blobs, gtt_blobs, inv_cnts = _prep_edges(edge_index)
    w = _prep_weights(inputs)

    key = "k"
    if key not in _CACHE:
        nc = _build(meta)
        _CACHE[key] = (_Runner(nc), meta)
    runner, _ = _CACHE[key]

    in_maps = []
    for c in range(N_CORES):
        xT_c = np.zeros((IN_DIM, NP), BF16)
        xT_c[:, :NC] = x[c * NC:(c + 1) * NC, :].T.astype(BF16)
        in_maps.append({
            "xT": xT_c, "W1T": w["W1T"], "R1": w["R1"], "bias1": w["bias1"],
            "R2": w["R2"], "bias2": w["bias2"], "WfcT": w["WfcT"],
            "bfc": w["bfc"], "b1": w["b1"],
            "idxb": idx_blobs[c], "gtb": gt_blobs[c], "gttb": gtt_blobs[c],
            "invc": inv_cnts[c],
        })
    results, _ = runner.run(in_maps)
    logits = np.concatenate(
        [results[c]["outT"][:, :NC].T for c in range(N_CORES)], axis=0
    ).astype(np.float32)
    return (logits, logits)



# revision 18
# speedup vs baseline: 2.9923x; 2.9923x over previous
"""Trainium2 Bass kernel for CustomStellarModel2 (GNN message passing).

Self-contained: host-side sharding/preprocessing + Bass/Tile kernel
compiled and run on 8 NeuronCores via PJRT (axon), then unsharded.

Strategy:
  - Nodes sharded contiguously across 8 cores (12500/core, padded to 12544).
  - Edges partitioned by dst owner, grouped by dst-block of 128 nodes,
    sorted by src, split into 4 banks of <=32768 rows so src indices fit
    int16 for dma_gather.
  - Per-edge gathers of src-node features (xl in layer 1, q|v in layer 2)
    via dma_gather on 4 SWDGE queues.
  - Dst-side per-edge expansion (gamma/beta, k) and segment reduction done
    with host-precomputed one-hot matrices (fp8, exact) on TensorE; the
    segment-sum accumulates in PSUM. No scatter needed.
  - Two AllGather collectives replicate the gather tables (xl, qv).
"""
import math
import numpy as np
import ml_dtypes

BF16 = ml_dtypes.bfloat16
FP8 = ml_dtypes.float8_e4m3

# Problem constants (hardcoded per contest rules); _config() allows
# small-scale overrides for simulator testing.
IN_DIM, H, C_OUT = 64, 128, 20
N_CORES = 8


def _config(n=100000, e=1600000, bank=32768, gcap=1024):
    global N, E, NC, G, NP, NFULL, BANK, N_BANKS, GCAP
    N, E, BANK, GCAP = n, e, bank, gcap
    NC = N // N_CORES            # real nodes per core
    G = math.ceil(NC / 128)      # groups per core
    NP = G * 128                 # padded nodes per core
    NFULL = NP * N_CORES         # padded table rows
    N_BANKS = math.ceil(NFULL / BANK)


_config()
ABLATE = set()


# ---------------------------------------------------------------------------
# Host-side preprocessing
# ---------------------------------------------------------------------------

def _prep_edges(edge_index):
    """Partition/sort/pad edges; build common (cross-core) schedule,
    per-core idx blobs, Gt/GtT one-hot blobs, and inv_cnt."""
    src = edge_index[0].astype(np.int64)
    dst = edge_index[1].astype(np.int64)
    core_of = dst // NC
    per_core = []
    for c in range(N_CORES):
        m = core_of == c
        s, d = src[m], dst[m] - c * NC
        # padded global row id of source node
        sc = s // NC
        s_pad = sc * NP + (s - sc * NC)
        g = d // 128
        bank = s_pad // BANK
        order = np.lexsort((s_pad, bank, g))
        per_core.append((s_pad[order], d[order], g[order], bank[order]))

    # counts per (core, group, bank)
    cnts = np.zeros((N_CORES, G, N_BANKS), np.int64)
    for c in range(N_CORES):
        _, _, g, b = per_core[c]
        np.add.at(cnts, (c, g, b), 1)
    # common schedule: pad to max over cores, round to 128, split into <=GCAP
    npad = ((cnts.max(axis=0) + 127) // 128) * 128  # [G, N_BANKS]
    # per-(g,b) gather instructions: list of idx counts (multiples of 128)
    sched = []  # per group: list of (bank, n_idx, col_off, g, b, j0)
    Cg = np.zeros(G, np.int64)
    n_items = 0
    for g in range(G):
        items = []
        col = 0
        for b in range(N_BANKS):
            n = int(npad[g, b])
            while n > 0:
                take = min(n, GCAP)
                items.append((b, take, col, g, b, int(npad[g, b]) - n))
                col += take // 128
                n -= take
                n_items += 1
        sched.append(items)
        Cg[g] = col
    TOTC = int(Cg.sum())
    # per-core real gather counts per sched item (>=1: zero-count chunks
    # gather row 0 into a pad slot, killed by the zero one-hot column)
    gcnts = []
    for c in range(N_CORES):
        gc = np.zeros(n_items, np.int32)
        it = 0
        for g in range(G):
            for (b, take, col, _, _, j0) in sched[g]:
                real = int(min(max(cnts[c, g, b] - j0, 0), take))
                gc[it] = max(real, 1)
                it += 1
        gcnts.append(gc[None, :])  # [1, n_items]
    meta_items = n_items
    g_coff = np.zeros(G + 1, np.int64)
    g_coff[1:] = np.cumsum(Cg)

    # per-core blobs
    idx_blobs, gt_blobs, gtt_blobs = [], [], []
    inv_cnts = []
    for c in range(N_CORES):
        s_pad, d, g, b = per_core[c]
        # in-degree of real nodes (for mean)
        cnt = np.zeros(NP, np.float32)
        np.add.at(cnt, d, 1.0)
        inv = 1.0 / np.maximum(cnt, 1.0)
        inv_cnts.append(inv.reshape(G, 128).T.copy())  # [128, G]

        # slot assignment: for each (g, bank), edges fill columns in order
        idx16 = np.zeros((128, TOTC * 128 // 16), np.int16)
        gt = np.zeros((128, TOTC * 128), FP8)
        gtt = np.zeros((128, TOTC * 128), FP8)
        # build per-(g,b) runs
        # edges already sorted by (g, bank, src)
        run_starts = np.zeros((G, N_BANKS), np.int64)
        np.cumsum(cnts[c].reshape(-1))
        flat = np.concatenate([[0], np.cumsum(cnts[c].reshape(-1))])
        run_starts = flat[:-1].reshape(G, N_BANKS)
        for gi in range(G):
            colbase = g_coff[gi]
            boff = 0
            for bi in range(N_BANKS):
                n_real = int(cnts[c, gi, bi])
                n_p = int(npad[gi, bi])
                e0 = int(run_starts[gi, bi])
                rel = np.full(n_p, -1, np.int64)  # -1 => pad (gather skips)
                drel = np.full(n_p, 128, np.int64)  # 128 => padding
                rel[:n_real] = s_pad[e0:e0 + n_real] - bi * BANK
                for j0_ in range(0, n_p, GCAP):  # no all-pad chunks
                    if n_real <= j0_:
                        rel[j0_] = 0
                drel[:n_real] = d[e0:e0 + n_real] - gi * 128
                # slot i -> (p=i%128, col=colbase+boff+i//128)
                i = np.arange(n_p)
                p = i % 128
                col = colbase + boff + i // 128
                # idx blob: idx i within one gather instr at [i%16, w0+i//16]
                # instructions of GCAP idxs starting at boff
                j = 0
                while j < n_p:
                    take = min(n_p - j, GCAP)
                    w0 = (colbase + boff) * 8 + j // 16
                    ii = np.arange(take)
                    blk = np.zeros((16, (take + 15) // 16), np.int16)
                    blk[ii % 16, ii // 16] = rel[j:j + take].astype(np.int16)
                    for rep in range(8):
                        idx16[rep * 16:(rep + 1) * 16, w0:w0 + take // 16] = blk
                    j += take
                # one-hot fills (skip padding slots)
                mreal = drel < 128
                pp, cc, ss = p[mreal], col[mreal], drel[mreal]
                gt[pp, cc * 128 + ss] = 1.0
                gtt[ss, cc * 128 + pp] = 1.0
                boff += n_p // 128
        # pack [idx | gt | gtt] per group into one uint8 blob (1 DMA/group)
        blob = np.zeros((128, TOTC * 272), np.uint8)
        for gi in range(G):
            co, Cg_i = int(g_coff[gi]), int(Cg[gi])
            b0 = co * 272
            blob[:, b0:b0 + Cg_i * 16] = \
                idx16[:, co * 8:(co + Cg_i) * 8].view(np.uint8)
            blob[:, b0 + Cg_i * 16:b0 + Cg_i * 144] = \
                gt[:, co * 128:(co + Cg_i) * 128].view(np.uint8)
            blob[:, b0 + Cg_i * 144:b0 + Cg_i * 272] = \
                gtt[:, co * 128:(co + Cg_i) * 128].view(np.uint8)
        idx_blobs.append(blob)
        gt_blobs.append(gt)
        gtt_blobs.append(gtt)

    meta = {
        "sched": sched, "Cg": Cg.astype(int), "g_coff": g_coff, "TOTC": TOTC,
        "n_items": meta_items,
    }
    return meta, idx_blobs, gt_blobs, gtt_blobs, inv_cnts, gcnts


def _prep_weights(inp):
    f = lambda a: np.ascontiguousarray(a, dtype=np.float32)
    W1T = f(inp["W1"]).T.astype(BF16)                      # [64,128]
    # reference: beta = bg[:, :H], gamma = bg[:, H:].  The kernel consumes
    # [gamma | beta] column order, so swap halves here.
    WfT = f(inp["Wf"]).T
    WfT_gb = np.concatenate([WfT[:, H:], WfT[:, :H]], axis=1)
    bf_gb = np.concatenate([f(inp["bf"])[H:], f(inp["bf"])[:H]])
    WfsT = f(inp["Wfs"]).T
    WfsT_gb = np.concatenate([WfsT[:, H:], WfsT[:, :H]], axis=1)
    bfs_gb = np.concatenate([f(inp["bfs"])[H:], f(inp["bfs"])[:H]])
    R1 = np.concatenate(
        [f(inp["Wl"]).T, WfT_gb, WfsT_gb, f(inp["Wls"]).T],
        axis=1).astype(BF16)                               # [128, 768]
    bias1 = np.concatenate(
        [np.zeros(H, np.float32), bf_gb, bfs_gb,
         np.zeros(H, np.float32)])[None, :].astype(BF16)   # [1, 768]
    R2 = np.concatenate(
        [f(inp["Wk"]).T, f(inp["Wskip"]).T, f(inp["Wq"]).T, f(inp["Wv"]).T],
        axis=1).astype(BF16)                               # [128, 512]
    bias2 = np.concatenate(
        [f(inp["bk"]), f(inp["bres"]), f(inp["bq"]), f(inp["bv"])]
    )[None, :].astype(BF16)                                # [1, 512]
    WfcT = f(inp["Wfc"]).T.astype(BF16)                    # [128, 20]
    bfc = f(inp["bfc"])[None, :].astype(BF16)              # [1, 20]
    b1 = f(inp["b1"])[None, :].astype(BF16)                # [1, 128]
    return dict(W1T=W1T, R1=R1, bias1=bias1, R2=R2, bias2=bias2,
                WfcT=WfcT, bfc=bfc, b1=b1)


# ---------------------------------------------------------------------------
# Bass kernel builder
# ---------------------------------------------------------------------------

def _build(meta):
    import concourse.bass as bass
    import concourse.bacc as bacc
    import concourse.mybir as mybir
    import concourse.tile as tile
    from concourse import library_config
    from concourse.masks import make_identity

    dt = mybir.dt
    sched, Cg, g_coff, TOTC = (meta["sched"], meta["Cg"], meta["g_coff"],
                               meta["TOTC"])
    CMAX = int(max(Cg))

    nc = bacc.Bacc("TRN2", target_bir_lowering=False, debug=False,
                   num_devices=N_CORES, dynamic_dma_scratch_size=131072,
                   num_swdge_queues=4)

    # ---- external inputs ----
    xT = nc.dram_tensor("xT", [IN_DIM, NP], dt.bfloat16,
                        kind="ExternalInput").ap()
    W1T = nc.dram_tensor("W1T", [IN_DIM, H], dt.bfloat16,
                         kind="ExternalInput").ap()
    R1 = nc.dram_tensor("R1", [H, 6 * H], dt.bfloat16,
                        kind="ExternalInput").ap()
    bias1 = nc.dram_tensor("bias1", [1, 6 * H], dt.bfloat16,
                           kind="ExternalInput").ap()
    R2 = nc.dram_tensor("R2", [H, 4 * H], dt.bfloat16,
                        kind="ExternalInput").ap()
    bias2 = nc.dram_tensor("bias2", [1, 4 * H], dt.bfloat16,
                           kind="ExternalInput").ap()
    WfcT = nc.dram_tensor("WfcT", [H, C_OUT], dt.bfloat16,
                          kind="ExternalInput").ap()
    bfc = nc.dram_tensor("bfc", [1, C_OUT], dt.bfloat16,
                         kind="ExternalInput").ap()
    b1 = nc.dram_tensor("b1", [1, H], dt.bfloat16, kind="ExternalInput").ap()
    blobd = nc.dram_tensor("blobd", [128, TOTC * 272], dt.uint8,
                           kind="ExternalInput").ap()
    gcntd = nc.dram_tensor("gcnt", [1, meta["n_items"]], dt.int32,
                           kind="ExternalInput").ap()
    invc = nc.dram_tensor("invc", [128, G], dt.float32,
                          kind="ExternalInput").ap()
    outT = nc.dram_tensor("outT", [C_OUT, NP], dt.float32,
                          kind="ExternalOutput").ap()
    xl_full = nc.dram_tensor("xl_full_sh", [NFULL, H], dt.bfloat16,
                             kind="Internal", addr_space="Shared").ap()
    qv_full = nc.dram_tensor("qv_full_sh", [NFULL, 2 * H], dt.bfloat16,
                             kind="Internal", addr_space="Shared").ap()

    with tile.TileContext(nc) as tc:
        with (
            tc.tile_pool(name="dram", bufs=1, space="DRAM") as dp,
            tc.tile_pool(name="const", bufs=1) as cp,
        ):
            nc.gpsimd.load_library(library_config.mlp)
            # DRAM intermediates
            xl_loc = dp.tile([NP, H], dt.bfloat16)
            gb_loc = dp.tile([NP, 2 * H], dt.bfloat16)
            skip_loc = dp.tile([NP, H], dt.bfloat16)
            x2T_loc = dp.tile([G, 128, H], dt.bfloat16)
            k_loc = dp.tile([NP, H], dt.bfloat16)
            skip2_loc = dp.tile([NP, H], dt.bfloat16)
            qv_loc = dp.tile([NP, 2 * H], dt.bfloat16)

            # constants in SBUF
            ones_col = cp.tile([1, 512], dt.bfloat16)
            nc.vector.memset(ones_col[:], 1.0)
            ident = cp.tile([128, 128], dt.bfloat16)
            make_identity(nc, ident[:])
            w1t_t = cp.tile([IN_DIM, H], dt.bfloat16)
            nc.sync.dma_start(out=w1t_t[:], in_=W1T[:])
            r1_t = cp.tile([H, 6 * H], dt.bfloat16)
            nc.sync.dma_start(out=r1_t[:], in_=R1[:])
            bias1_t = cp.tile([1, 6 * H], dt.bfloat16)
            nc.sync.dma_start(out=bias1_t[:], in_=bias1[:])
            r2_t = cp.tile([H, 4 * H], dt.bfloat16)
            nc.sync.dma_start(out=r2_t[:], in_=R2[:])
            bias2_t = cp.tile([1, 4 * H], dt.bfloat16)
            nc.sync.dma_start(out=bias2_t[:], in_=bias2[:])
            wfc_t = cp.tile([H, C_OUT], dt.bfloat16)
            nc.sync.dma_start(out=wfc_t[:], in_=WfcT[:])
            bfc_t = cp.tile([1, C_OUT], dt.bfloat16)
            nc.sync.dma_start(out=bfc_t[:], in_=bfc[:])
            b1_t = cp.tile([1, H], dt.bfloat16)
            nc.sync.dma_start(out=b1_t[:], in_=b1[:])
            invc_t = cp.tile([128, G], dt.float32)
            nc.sync.dma_start(out=invc_t[:], in_=invc[:])
            gcnt_t = cp.tile([1, meta["n_items"]], dt.int32)
            nc.sync.dma_start(out=gcnt_t[:], in_=gcntd[:])
            item_base = np.zeros(G + 1, np.int64)
            for g in range(G):
                item_base[g + 1] = item_base[g] + len(sched[g])

            # ---------------- P1: dense layer 1 (local nodes) -------------
            with (
                tc.tile_pool(name="p1sb", bufs=3) as sb,
                tc.tile_pool(name="p1ps", bufs=2, space="PSUM") as ps,
                tc.tile_pool(name="p1ps2", bufs=2, space="PSUM") as ps2,
            ):
                NB = 512
                for s0 in range(0, NP, NB):
                    n = min(NB, NP - s0)
                    xt_t = sb.tile([IN_DIM, n], dt.bfloat16, tag="xt")
                    nc.sync.dma_start(out=xt_t[:], in_=xT[:, s0:s0 + n])
                    p1 = ps.tile([128, NB], dt.float32, tag="p1")
                    nc.tensor.matmul(p1[:, :n], lhsT=w1t_t[:], rhs=xt_t[:],
                                     start=True, stop=False)
                    nc.tensor.matmul(p1[:, :n], lhsT=b1_t[:],
                                     rhs=ones_col[:, :n],
                                     start=False, stop=True)
                    x1t = sb.tile([128, NB], dt.bfloat16, tag="x1t")
                    nc.scalar.activation(x1t[:, :n], p1[:, :n],
                                         mybir.ActivationFunctionType.Relu)
                    for nb in range(0, n, 128):
                        m = min(128, n - nb)
                        p2 = ps2.tile([128, 6 * H], dt.float32, tag="p2")
                        lhsT = x1t[:, nb:nb + m]
                        nc.tensor.matmul(p2[:m, :512], lhsT=lhsT,
                                         rhs=r1_t[:, :512],
                                         start=True, stop=False)
                        nc.tensor.matmul(p2[:m, :512], lhsT=ones_col[:, :m],
                                         rhs=bias1_t[:, :512],
                                         start=False, stop=True)
                        nc.tensor.matmul(p2[:m, 512:], lhsT=lhsT,
                                         rhs=r1_t[:, 512:],
                                         start=True, stop=False)
                        nc.tensor.matmul(p2[:m, 512:], lhsT=ones_col[:, :m],
                                         rhs=bias1_t[:, 512:],
                                         start=False, stop=True)
                        row0 = s0 + nb
                        # xl and gamma-beta out
                        xlgb = sb.tile([128, 3 * H], dt.bfloat16, tag="xlgb")
                        nc.scalar.copy(xlgb[:m, :], p2[:m, :3 * H])
                        nc.sync.dma_start(out=xl_loc[row0:row0 + m, :],
                                          in_=xlgb[:m, :H])
                        nc.sync.dma_start(out=gb_loc[row0:row0 + m, :],
                                          in_=xlgb[:m, H:])
                        # FiLM skip: relu(gs * xls + bs)
                        # (copy PSUM slice to SBUF first: DVE tensor_tensor
                        # may read at most one PSUM operand)
                        sks = sb.tile([128, 3 * H], dt.bfloat16, tag="sks")
                        nc.scalar.copy(sks[:m, :], p2[:m, 384:768])
                        tmp = sb.tile([128, H], dt.bfloat16, tag="tmp")
                        nc.vector.tensor_mul(tmp[:m, :], sks[:m, :H],
                                             sks[:m, 2 * H:])
                        pre = sb.tile([128, H], dt.bfloat16, tag="pre")
                        nc.vector.tensor_add(pre[:m, :], tmp[:m, :],
                                             sks[:m, H:2 * H])
                        sk = sb.tile([128, H], dt.bfloat16, tag="sk")
                        nc.vector.tensor_scalar_max(sk[:m, :], pre[:m, :], 0.0)
                        nc.sync.dma_start(out=skip_loc[row0:row0 + m, :],
                                          in_=sk[:m, :])

            # ---------------- P2: AllGather xl ----------------------------
            nc.gpsimd.collective_compute(
                "AllGather", mybir.AluOpType.bypass,
                replica_groups=[list(range(N_CORES))],
                ins=[xl_loc[:]], outs=[xl_full[:]],
            )

            # ---------------- P3: FiLM edge phase -------------------------
            with (
                tc.tile_pool(name="p3sb", bufs=2) as sb,
                tc.tile_pool(name="p3g", bufs=2) as gpool,
                tc.tile_pool(name="p3ps", bufs=2, space="PSUM") as pse,
                tc.tile_pool(name="p3pa", bufs=2, space="PSUM") as psa,
                tc.tile_pool(name="p3pt", bufs=2, space="PSUM") as pst,
            ):
                for g in range(G):
                    C = int(Cg[g])
                    co = int(g_coff[g])
                    gb_g = sb.tile([128, 2 * H], dt.bfloat16, tag="gbg")
                    nc.sync.dma_start(out=gb_g[:],
                                      in_=gb_loc[g * 128:(g + 1) * 128, :])
                    blob_g = sb.tile([128, CMAX * 272], dt.uint8, tag="blob")
                    nc.sync.dma_start(out=blob_g[:, :C * 272],
                                      in_=blobd[:, co * 272:(co + C) * 272])
                    idx_g = blob_g[:, :C * 16].bitcast(dt.int16)
                    gt_g = blob_g[:, C * 16:C * 144].bitcast(dt.float8e4)
                    gtt_g = blob_g[:, C * 144:C * 272].bitcast(dt.float8e4)
                    skp_g = sb.tile([128, H], dt.bfloat16, tag="skp")
                    nc.sync.dma_start(out=skp_g[:],
                                      in_=skip_loc[g * 128:(g + 1) * 128, :])
                    # gathers (-1 idx pads are skipped by SWDGE)
                    xg = gpool.tile([128, CMAX, H], dt.bfloat16, tag="xg")
                    if g < 2:
                        nc.vector.memset(xg[:], 0.0)
                    for qi, (b, nidx, coloff, _, _, _) in enumerate(sched[g]):
                        nrow = min(BANK, NFULL - b * BANK)
                        it = int(item_base[g]) + qi
                        with nc.gpsimd.register() as reg:
                            nc.gpsimd.reg_load(reg, gcnt_t[0:1, it:it + 1])
                            nc.gpsimd.dma_gather(
                                xg[:, coloff:coloff + nidx // 128, :],
                                xl_full[b * BANK:b * BANK + nrow, :],
                                idx_g[:, coloff * 8:coloff * 8 + nidx // 16],
                                nidx, reg, H, queue_num=qi % 4,
                            )
                    # expansion of gamma/beta to edges (4-chunk batches);
                    # msg = relu(ge * xg + be) with DVE reading PSUM directly
                    m1 = gpool.tile([128, CMAX, H], dt.bfloat16, tag="m1")
                    for c0 in range(0, C, 4):
                        cn = min(4, C - c0)
                        pe = pse.tile([128, 4, 2 * H], dt.float32, tag="pe")
                        for j in range(cn):
                            c = c0 + j
                            nc.tensor.matmul(
                                pe[:, j, :],
                                lhsT=gtt_g[:, c * 128:(c + 1) * 128],
                                rhs=gb_g[:], start=True, stop=True)
                        m0 = gpool.tile([128, 4, H], dt.bfloat16, tag="m0")
                        nc.vector.tensor_mul(m0[:, :cn, :], pe[:, :cn, :H],
                                             xg[:, c0:c0 + cn, :])
                        nc.vector.tensor_add(m1[:, c0:c0 + cn, :],
                                             m0[:, :cn, :], pe[:, :cn, H:])
                    msg = gpool.tile([128, CMAX, H], dt.bfloat16, tag="msg")
                    nc.vector.tensor_scalar_max(msg[:, :C, :],
                                                m1[:, :C, :], 0.0)
                    # segment reduce via Gt matmuls
                    pa = psa.tile([128, H], dt.float32, tag="pa")
                    for c in (range(1) if "reduce" in ABLATE else range(C)):
                        nc.tensor.matmul(pa[:],
                                         lhsT=gt_g[:, c * 128:(c + 1) * 128],
                                         rhs=msg[:, c, :],
                                         start=(c == 0),
                                         stop=(c == C - 1
                                               or "reduce" in ABLATE))
                    # x2 = relu(skip + agg/cnt)
                    aggm = sb.tile([128, H], dt.float32, tag="aggm")
                    nc.vector.tensor_scalar_mul(aggm[:], pa[:],
                                                invc_t[:, g:g + 1])
                    x2p = sb.tile([128, H], dt.float32, tag="x2p")
                    nc.vector.tensor_add(x2p[:], aggm[:], skp_g[:])
                    x2 = sb.tile([128, H], dt.bfloat16, tag="x2")
                    nc.vector.tensor_scalar_max(x2[:], x2p[:], 0.0)
                    # x2 transpose for dense-2 lhsT
                    pt = pst.tile([128, H], dt.bfloat16, tag="pt")
                    nc.tensor.transpose(pt[:], x2[:], ident[:])
                    x2t = sb.tile([128, H], dt.bfloat16, tag="x2t")
                    nc.scalar.copy(x2t[:], pt[:])
                    nc.sync.dma_start(out=x2T_loc[g, :, :], in_=x2t[:])

            # ---------------- P4: dense layer 2 (local) -------------------
            with (
                tc.tile_pool(name="p4sb", bufs=3) as sb,
                tc.tile_pool(name="p4ps", bufs=2, space="PSUM") as ps,
            ):
                for g in range(G):
                    x2t = sb.tile([128, H], dt.bfloat16, tag="x2t")
                    nc.sync.dma_start(out=x2t[:], in_=x2T_loc[g, :, :])
                    p2 = ps.tile([128, 4 * H], dt.float32, tag="p2")
                    nc.tensor.matmul(p2[:], lhsT=x2t[:], rhs=r2_t[:],
                                     start=True, stop=False)
                    nc.tensor.matmul(p2[:], lhsT=ones_col[:, :128],
                                     rhs=bias2_t[:], start=False, stop=True)
                    kk = sb.tile([128, 2 * H], dt.bfloat16, tag="kk")
                    nc.scalar.copy(kk[:], p2[:, :2 * H])
                    row0 = g * 128
                    nc.sync.dma_start(out=k_loc[row0:row0 + 128, :],
                                      in_=kk[:, :H])
                    nc.sync.dma_start(out=skip2_loc[row0:row0 + 128, :],
                                      in_=kk[:, H:])
                    qv = sb.tile([128, 2 * H], dt.bfloat16, tag="qv")
                    nc.scalar.copy(qv[:], p2[:, 2 * H:])
                    nc.sync.dma_start(out=qv_loc[row0:row0 + 128, :],
                                      in_=qv[:])

            # ---------------- P5: AllGather qv ----------------------------
            nc.gpsimd.collective_compute(
                "AllGather", mybir.AluOpType.bypass,
                replica_groups=[list(range(N_CORES))],
                ins=[qv_loc[:]], outs=[qv_full[:]],
            )

            # ---------------- P6: ResGated edge phase ---------------------
            with (
                tc.tile_pool(name="p6sb", bufs=2) as sb,
                tc.tile_pool(name="p6g", bufs=2) as gpool,
                tc.tile_pool(name="p6ps", bufs=2, space="PSUM") as psk,
                tc.tile_pool(name="p6pa", bufs=2, space="PSUM") as psa,
                tc.tile_pool(name="p6pt", bufs=2, space="PSUM") as pst,
                tc.tile_pool(name="p6pl", bufs=2, space="PSUM") as psl,
            ):
                for g in range(G):
                    C = int(Cg[g])
                    co = int(g_coff[g])
                    k_g = sb.tile([128, H], dt.bfloat16, tag="kg")
                    nc.sync.dma_start(out=k_g[:],
                                      in_=k_loc[g * 128:(g + 1) * 128, :])
                    sk2_g = sb.tile([128, H], dt.bfloat16, tag="sk2")
                    nc.sync.dma_start(out=sk2_g[:],
                                      in_=skip2_loc[g * 128:(g + 1) * 128, :])
                    blob_g = sb.tile([128, CMAX * 272], dt.uint8, tag="blob6")
                    nc.sync.dma_start(out=blob_g[:, :C * 272],
                                      in_=blobd[:, co * 272:(co + C) * 272])
                    idx_g = blob_g[:, :C * 16].bitcast(dt.int16)
                    gt_g = blob_g[:, C * 16:C * 144].bitcast(dt.float8e4)
                    gtt_g = blob_g[:, C * 144:C * 272].bitcast(dt.float8e4)
                    qvg = gpool.tile([128, CMAX, 2 * H], dt.bfloat16,
                                     tag="qvg")
                    if g < 2:
                        nc.vector.memset(qvg[:], 0.0)
                    for qi, (b, nidx, coloff, _, _, _) in enumerate(sched[g]):
                        nrow = min(BANK, NFULL - b * BANK)
                        it = int(item_base[g]) + qi
                        with nc.gpsimd.register() as reg:
                            nc.gpsimd.reg_load(reg, gcnt_t[0:1, it:it + 1])
                            nc.gpsimd.dma_gather(
                                qvg[:, coloff:coloff + nidx // 128, :],
                                qv_full[b * BANK:b * BANK + nrow, :],
                                idx_g[:, coloff * 8:coloff * 8 + nidx // 16],
                                nidx, reg, 2 * H, queue_num=qi % 4,
                            )
                    # expand k to edges; kq = ke + q (DVE reads PSUM)
                    kq = gpool.tile([128, CMAX, H], dt.bfloat16, tag="kq")
                    for c0 in range(0, C, 4):
                        cn = min(4, C - c0)
                        pk = psk.tile([128, 4, H], dt.float32, tag="pk")
                        for j in range(cn):
                            c = c0 + j
                            nc.tensor.matmul(
                                pk[:, j, :],
                                lhsT=gtt_g[:, c * 128:(c + 1) * 128],
                                rhs=k_g[:], start=True, stop=True)
                        nc.vector.tensor_add(kq[:, c0:c0 + cn, :],
                                             pk[:, :cn, :],
                                             qvg[:, c0:c0 + cn, :H])
                    # eta = sigmoid(kq); contrib = eta * vg
                    ctb = gpool.tile([128, CMAX, H], dt.bfloat16, tag="ctb")
                    eta = gpool.tile([128, CMAX, H], dt.bfloat16, tag="eta")
                    nc.scalar.activation(
                        eta[:, :C, :], kq[:, :C, :],
                        mybir.ActivationFunctionType.Sigmoid)
                    nc.vector.tensor_mul(ctb[:, :C, :], eta[:, :C, :],
                                         qvg[:, :C, H:])
                    pa = psa.tile([128, H], dt.float32, tag="pa6")
                    for c in (range(1) if "reduce" in ABLATE else range(C)):
                        nc.tensor.matmul(pa[:],
                                         lhsT=gt_g[:, c * 128:(c + 1) * 128],
                                         rhs=ctb[:, c, :],
                                         start=(c == 0),
                                         stop=(c == C - 1
                                               or "reduce" in ABLATE))
                    # x3 = relu(s2 + skip2)
                    x3p = sb.tile([128, H], dt.float32, tag="x3p")
                    nc.vector.tensor_add(x3p[:], pa[:], sk2_g[:])
                    x3 = sb.tile([128, H], dt.bfloat16, tag="x3")
                    nc.vector.tensor_scalar_max(x3[:], x3p[:], 0.0)
                    pt = pst.tile([128, H], dt.bfloat16, tag="pt6")
                    nc.tensor.transpose(pt[:], x3[:], ident[:])
                    x3t = sb.tile([128, H], dt.bfloat16, tag="x3t")
                    nc.scalar.copy(x3t[:], pt[:])
                    # logits_T = Wfc @ x3T + bfc
                    pl = psl.tile([C_OUT, 128], dt.float32, tag="pl")
                    nc.tensor.matmul(pl[:], lhsT=wfc_t[:], rhs=x3t[:],
                                     start=True, stop=False)
                    nc.tensor.matmul(pl[:], lhsT=bfc_t[:],
                                     rhs=ones_col[:, :128],
                                     start=False, stop=True)
                    lt = sb.tile([C_OUT, 128], dt.float32, tag="lt")
                    nc.scalar.copy(lt[:], pl[:])
                    nc.sync.dma_start(out=outT[:, g * 128:(g + 1) * 128],
                                      in_=lt[:])

    nc.compile()
    return nc


# ---------------------------------------------------------------------------
# Runner (PJRT shard_map, compile once)
# ---------------------------------------------------------------------------

class _Runner:
    def __init__(self, nc):
        import jax
        import concourse.mybir as mybir
        from concourse import bass2jax
        from concourse.bass2jax import _bass_exec_p, install_neuronx_cc_hook
        from jax.sharding import Mesh, PartitionSpec
        try:
            from jax.experimental.shard_map import shard_map
        except ImportError:
            from jax.sharding import shard_map
        install_neuronx_cc_hook()
        self.jax = jax
        partition_name = (nc.partition_id_tensor.name
                          if nc.partition_id_tensor else None)
        in_names, out_names, out_avals, zero_outs = [], [], [], []
        for alloc in nc.m.functions[0].allocations:
            if not isinstance(alloc, mybir.MemoryLocationSet):
                continue
            name = alloc.memorylocations[0].name
            if alloc.kind == "ExternalInput":
                if name != partition_name:
                    in_names.append(name)
            elif alloc.kind == "ExternalOutput":
                out_names.append(name)
                shape = tuple(alloc.tensor_shape)
                dtype = mybir.dt.np(alloc.dtype)
                out_avals.append(jax.core.ShapedArray(shape, dtype))
                zero_outs.append(np.zeros(shape, dtype))
        self.in_names, self.out_names = in_names, out_names
        self.out_avals, self.zero_outs = out_avals, zero_outs
        n_params, n_outs = len(in_names), len(out_avals)
        all_in = list(in_names) + list(out_names)
        if partition_name is not None:
            all_in.append(partition_name)

        def _body(*args):
            operands = list(args)
            if partition_name is not None:
                operands.append(bass2jax.partition_id_tensor())
            return tuple(_bass_exec_p.bind(
                *operands, out_avals=tuple(out_avals),
                in_names=tuple(all_in), out_names=tuple(out_names),
                lowering_input_output_aliases=(),
                sim_require_finite=True, sim_require_nnan=True, nc=nc))

        devices = jax.devices()[:N_CORES]
        self.mesh = Mesh(np.asarray(devices), ("core",))
        specs_in = (PartitionSpec("core"),) * (n_params + n_outs)
        specs_out = (PartitionSpec("core"),) * len(out_names)
        self.fn = jax.jit(
            shard_map(_body, mesh=self.mesh, in_specs=specs_in,
                      out_specs=specs_out, check_rep=False),
            keep_unused=True)

    def run(self, in_maps):
        jax = self.jax
        from jax.sharding import NamedSharding, PartitionSpec
        per_core = [[np.asarray(m[n]) for n in self.in_names]
                    for m in in_maps]
        concat = [np.concatenate([per_core[c][i] for c in range(N_CORES)], 0)
                  for i in range(len(self.in_names))]
        zeros = [np.zeros((N_CORES * z.shape[0], *z.shape[1:]), z.dtype)
                 for z in self.zero_outs]
        sh = NamedSharding(self.mesh, PartitionSpec("core"))
        args = [jax.device_put(a, sh) for a in concat + zeros]
        outs = self.fn(*args)
        jax.block_until_ready(outs)
        return [
            {n: np.asarray(outs[i]).reshape(N_CORES,
                                            *self.out_avals[i].shape)[c]
             for i, n in enumerate(self.out_names)}
            for c in range(N_CORES)
        ], (args, outs)


_CACHE = {}


def kernel(**inputs) -> np.ndarray:
    edge_index = np.asarray(inputs["edge_index"])
    x = np.asarray(inputs["x"], dtype=np.float32)

    meta, idx_blobs, gt_blobs, gtt_blobs, inv_cnts, gcnts = _prep_edges(
        edge_index)
    w = _prep_weights(inputs)

    key = "k"
    if key not in _CACHE:
        nc = _build(meta)
        _CACHE[key] = (_Runner(nc), meta)
    runner, _ = _CACHE[key]

    in_maps = []
    for c in range(N_CORES):
        xT_c = np.zeros((IN_DIM, NP), BF16)
        xT_c[:, :NC] = x[c * NC:(c + 1) * NC, :].T.astype(BF16)
        in_maps.append({
            "xT": xT_c, "W1T": w["W1T"], "R1": w["R1"], "bias1": w["bias1"],
            "R2": w["R2"], "bias2": w["bias2"], "WfcT": w["WfcT"],
            "bfc": w["bfc"], "b1": w["b1"],
            "blobd": idx_blobs[c], "invc": inv_cnts[c], "gcnt": gcnts[c],
        })
    results, _ = runner.run(in_maps)
    logits = np.concatenate(
        [results[c]["outT"][:, :NC].T for c in range(N_CORES)], axis=0
    ).astype(np.float32)
    return (logits, logits)



# revision 36
# speedup vs baseline: 3.9652x; 1.3251x over previous
"""Trainium2 Bass kernel for CustomStellarModel2 (GNN message passing).

Self-contained: host-side sharding/preprocessing + Bass/Tile kernel
compiled and run on 8 NeuronCores via PJRT (axon), then unsharded.

Strategy:
  - Nodes sharded contiguously across 8 cores (12500/core, padded to 12544).
  - Edges partitioned by dst owner, grouped by dst-block of 128 nodes,
    sorted by src, split into 4 banks of <=32768 rows so src indices fit
    int16 for dma_gather.
  - Per-edge gathers of src-node features (xl in layer 1, q|v in layer 2)
    via dma_gather on 4 SWDGE queues.
  - Dst-side per-edge expansion (gamma/beta, k) and segment reduction done
    with host-precomputed one-hot matrices (fp8, exact) on TensorE; the
    segment-sum accumulates in PSUM. No scatter needed.
  - Two AllGather collectives replicate the gather tables (xl, qv).
"""
import math
import numpy as np
import ml_dtypes

BF16 = ml_dtypes.bfloat16
FP8 = ml_dtypes.float8_e4m3

# Problem constants (hardcoded per contest rules); _config() allows
# small-scale overrides for simulator testing.
IN_DIM, H, C_OUT = 64, 128, 20
N_CORES = 8


def _config(n=100000, e=1600000, bank=32768, gcap=1024):
    global N, E, NC, G, NP, NFULL, BANK, N_BANKS, GCAP
    N, E, BANK, GCAP = n, e, bank, gcap
    NC = N // N_CORES            # real nodes per core
    G = math.ceil(NC / 128)      # groups per core
    NP = G * 128                 # padded nodes per core
    NFULL = NP * N_CORES         # padded table rows
    N_BANKS = math.ceil(NFULL / BANK)


_config()
ABLATE = set()


# ---------------------------------------------------------------------------
# Host-side preprocessing
# ---------------------------------------------------------------------------

def _prep_edges(edge_index):
    """Partition/sort/pad edges; build common (cross-core) schedule,
    per-core idx blobs, Gt/GtT one-hot blobs, and inv_cnt."""
    src = edge_index[0].astype(np.int64)
    dst = edge_index[1].astype(np.int64)
    core_of = dst // NC
    per_core = []
    for c in range(N_CORES):
        m = core_of == c
        s, d = src[m], dst[m] - c * NC
        # padded global row id of source node
        sc = s // NC
        s_pad = sc * NP + (s - sc * NC)
        g = d // 128
        bank = s_pad // BANK
        order = np.lexsort((s_pad, bank, g))
        per_core.append((s_pad[order], d[order], g[order], bank[order]))

    # counts per (core, group, bank)
    cnts = np.zeros((N_CORES, G, N_BANKS), np.int64)
    for c in range(N_CORES):
        _, _, g, b = per_core[c]
        np.add.at(cnts, (c, g, b), 1)
    # common schedule: pad to max over cores, round to 128, split into <=GCAP
    npad = ((cnts.max(axis=0) + 127) // 128) * 128  # [G, N_BANKS]
    # per-(g,b) gather instructions: list of idx counts (multiples of 128)
    sched = []  # per group: list of (bank, n_idx, col_off, g, b, j0)
    Cg = np.zeros(G, np.int64)
    n_items = 0
    for g in range(G):
        items = []
        col = 0
        for b in range(N_BANKS):
            n = int(npad[g, b])
            while n > 0:
                take = min(n, GCAP)
                items.append((b, take, col, g, b, int(npad[g, b]) - n))
                col += take // 128
                n -= take
                n_items += 1
        sched.append(items)
        Cg[g] = col
    TOTC = int(Cg.sum())
    # per-core real gather counts per sched item (>=1: zero-count chunks
    # gather row 0 into a pad slot, killed by the zero one-hot column)
    gcnts = []
    for c in range(N_CORES):
        gc = np.zeros(n_items, np.int32)
        it = 0
        for g in range(G):
            for (b, take, col, _, _, j0) in sched[g]:
                real = int(min(max(cnts[c, g, b] - j0, 0), take))
                gc[it] = max(real, 1)
                it += 1
        gcnts.append(gc[None, :])  # [1, n_items]
    meta_items = n_items
    g_coff = np.zeros(G + 1, np.int64)
    g_coff[1:] = np.cumsum(Cg)

    # per-core blobs
    idx_blobs, gt_blobs, gtt_blobs = [], [], []
    inv_cnts = []
    for c in range(N_CORES):
        s_pad, d, g, b = per_core[c]
        # in-degree of real nodes (for mean)
        cnt = np.zeros(NP, np.float32)
        np.add.at(cnt, d, 1.0)
        inv = 1.0 / np.maximum(cnt, 1.0)
        inv_cnts.append(inv.reshape(G, 128).T.copy())  # [128, G]

        # slot assignment: for each (g, bank), edges fill columns in order
        idx16 = np.zeros((128, TOTC * 128 // 16), np.int16)
        gt = np.zeros((128, TOTC * 128), FP8)
        gtt = np.zeros((128, TOTC * 128), FP8)
        # build per-(g,b) runs
        # edges already sorted by (g, bank, src)
        run_starts = np.zeros((G, N_BANKS), np.int64)
        np.cumsum(cnts[c].reshape(-1))
        flat = np.concatenate([[0], np.cumsum(cnts[c].reshape(-1))])
        run_starts = flat[:-1].reshape(G, N_BANKS)
        for gi in range(G):
            colbase = g_coff[gi]
            boff = 0
            for bi in range(N_BANKS):
                n_real = int(cnts[c, gi, bi])
                n_p = int(npad[gi, bi])
                e0 = int(run_starts[gi, bi])
                rel = np.full(n_p, -1, np.int64)  # -1 => pad (gather skips)
                drel = np.full(n_p, 128, np.int64)  # 128 => padding
                rel[:n_real] = s_pad[e0:e0 + n_real] - bi * BANK
                for j0_ in range(0, n_p, GCAP):  # no all-pad chunks
                    if n_real <= j0_:
                        rel[j0_] = 0
                drel[:n_real] = d[e0:e0 + n_real] - gi * 128
                # slot i -> (p=i%128, col=colbase+boff+i//128)
                i = np.arange(n_p)
                p = i % 128
                col = colbase + boff + i // 128
                # idx blob: idx i within one gather instr at [i%16, w0+i//16]
                # instructions of GCAP idxs starting at boff
                j = 0
                while j < n_p:
                    take = min(n_p - j, GCAP)
                    w0 = (colbase + boff) * 8 + j // 16
                    ii = np.arange(take)
                    blk = np.zeros((16, (take + 15) // 16), np.int16)
                    blk[ii % 16, ii // 16] = rel[j:j + take].astype(np.int16)
                    for rep in range(8):
                        idx16[rep * 16:(rep + 1) * 16, w0:w0 + take // 16] = blk
                    j += take
                # one-hot fills (skip padding slots)
                mreal = drel < 128
                pp, cc, ss = p[mreal], col[mreal], drel[mreal]
                gt[pp, cc * 128 + ss] = 1.0
                gtt[ss, cc * 128 + pp] = 1.0
                boff += n_p // 128
        # pack [idx | gt | gtt] per group into one uint8 blob (1 DMA/group)
        blob = np.zeros((128, TOTC * 272), np.uint8)
        for gi in range(G):
            co, Cg_i = int(g_coff[gi]), int(Cg[gi])
            b0 = co * 272
            blob[:, b0:b0 + Cg_i * 16] = \
                idx16[:, co * 8:(co + Cg_i) * 8].view(np.uint8)
            blob[:, b0 + Cg_i * 16:b0 + Cg_i * 144] = \
                gt[:, co * 128:(co + Cg_i) * 128].view(np.uint8)
            blob[:, b0 + Cg_i * 144:b0 + Cg_i * 272] = \
                gtt[:, co * 128:(co + Cg_i) * 128].view(np.uint8)
        idx_blobs.append(blob)
        gt_blobs.append(gt)
        gtt_blobs.append(gtt)

    meta = {
        "sched": sched, "Cg": Cg.astype(int), "g_coff": g_coff, "TOTC": TOTC,
        "n_items": meta_items,
    }
    return meta, idx_blobs, gt_blobs, gtt_blobs, inv_cnts, gcnts


def _prep_weights(inp):
    f = lambda a: np.ascontiguousarray(a, dtype=np.float32)
    W1T = f(inp["W1"]).T.astype(BF16)                      # [64,128]
    # reference: beta = bg[:, :H], gamma = bg[:, H:].  The kernel consumes
    # [gamma | beta] column order, so swap halves here.
    WfT = f(inp["Wf"]).T
    WfT_gb = np.concatenate([WfT[:, H:], WfT[:, :H]], axis=1)
    bf_gb = np.concatenate([f(inp["bf"])[H:], f(inp["bf"])[:H]])
    WfsT = f(inp["Wfs"]).T
    WfsT_gb = np.concatenate([WfsT[:, H:], WfsT[:, :H]], axis=1)
    bfs_gb = np.concatenate([f(inp["bfs"])[H:], f(inp["bfs"])[:H]])
    R1 = np.concatenate(
        [f(inp["Wl"]).T, WfT_gb, WfsT_gb, f(inp["Wls"]).T],
        axis=1).astype(BF16)                               # [128, 768]
    bias1 = np.concatenate(
        [np.zeros(H, np.float32), bf_gb, bfs_gb,
         np.zeros(H, np.float32)])[None, :].astype(BF16)   # [1, 768]
    R2 = np.concatenate(
        [f(inp["Wk"]).T, f(inp["Wskip"]).T, f(inp["Wq"]).T, f(inp["Wv"]).T],
        axis=1).astype(BF16)                               # [128, 512]
    bias2 = np.concatenate(
        [f(inp["bk"]), f(inp["bres"]), f(inp["bq"]), f(inp["bv"])]
    )[None, :].astype(BF16)                                # [1, 512]
    WfcT = f(inp["Wfc"]).T.astype(BF16)                    # [128, 20]
    bfc = f(inp["bfc"])[None, :].astype(BF16)              # [1, 20]
    b1 = f(inp["b1"])[None, :].astype(BF16)                # [1, 128]
    return dict(W1T=W1T, R1=R1, bias1=bias1, R2=R2, bias2=bias2,
                WfcT=WfcT, bfc=bfc, b1=b1)


# ---------------------------------------------------------------------------
# Bass kernel builder
# ---------------------------------------------------------------------------

def _build(meta):
    import concourse.bass as bass
    import concourse.bacc as bacc
    import concourse.mybir as mybir
    import concourse.tile as tile
    from concourse import library_config
    from concourse.masks import make_identity

    dt = mybir.dt
    sched, Cg, g_coff, TOTC = (meta["sched"], meta["Cg"], meta["g_coff"],
                               meta["TOTC"])
    CMAX = int(max(Cg))

    nc = bacc.Bacc("TRN2", target_bir_lowering=False, debug=False,
                   num_devices=N_CORES, dynamic_dma_scratch_size=131072,
                   num_swdge_queues=4)

    # ---- external inputs ----
    xT = nc.dram_tensor("xT", [IN_DIM, NP], dt.bfloat16,
                        kind="ExternalInput").ap()
    W1T = nc.dram_tensor("W1T", [IN_DIM, H], dt.bfloat16,
                         kind="ExternalInput").ap()
    R1 = nc.dram_tensor("R1", [H, 6 * H], dt.bfloat16,
                        kind="ExternalInput").ap()
    bias1 = nc.dram_tensor("bias1", [1, 6 * H], dt.bfloat16,
                           kind="ExternalInput").ap()
    R2 = nc.dram_tensor("R2", [H, 4 * H], dt.bfloat16,
                        kind="ExternalInput").ap()
    bias2 = nc.dram_tensor("bias2", [1, 4 * H], dt.bfloat16,
                           kind="ExternalInput").ap()
    WfcT = nc.dram_tensor("WfcT", [H, C_OUT], dt.bfloat16,
                          kind="ExternalInput").ap()
    bfc = nc.dram_tensor("bfc", [1, C_OUT], dt.bfloat16,
                         kind="ExternalInput").ap()
    b1 = nc.dram_tensor("b1", [1, H], dt.bfloat16, kind="ExternalInput").ap()
    blobd = nc.dram_tensor("blobd", [128, TOTC * 272], dt.uint8,
                           kind="ExternalInput").ap()
    gcntd = nc.dram_tensor("gcnt", [1, meta["n_items"]], dt.int32,
                           kind="ExternalInput").ap()
    invc = nc.dram_tensor("invc", [128, G], dt.float32,
                          kind="ExternalInput").ap()
    outT = nc.dram_tensor("outT", [C_OUT, NP], dt.float32,
                          kind="ExternalOutput").ap()
    xl_full = nc.dram_tensor("xl_full_sh", [NFULL, H], dt.bfloat16,
                             kind="Internal", addr_space="Shared").ap()
    qv_full = nc.dram_tensor("qv_full_sh", [NFULL, 2 * H], dt.bfloat16,
                             kind="Internal", addr_space="Shared").ap()

    with tile.TileContext(nc) as tc:
        with (
            tc.tile_pool(name="dram", bufs=1, space="DRAM") as dp,
            tc.tile_pool(name="const", bufs=1) as cp,
        ):
            nc.gpsimd.load_library(library_config.mlp)
            # DRAM intermediates
            xl_loc = dp.tile([NP, H], dt.bfloat16)
            gb_loc = dp.tile([NP, 2 * H], dt.bfloat16)
            skip_loc = dp.tile([NP, H], dt.bfloat16)
            skip2_loc = dp.tile([NP, H], dt.bfloat16)
            qv_loc = dp.tile([NP, 2 * H], dt.bfloat16)

            # constants in SBUF
            ones_col = cp.tile([1, 512], dt.bfloat16)
            nc.vector.memset(ones_col[:], 1.0)
            ident = cp.tile([128, 128], dt.bfloat16)
            make_identity(nc, ident[:])
            w1t_t = cp.tile([IN_DIM, H], dt.bfloat16)
            nc.sync.dma_start(out=w1t_t[:], in_=W1T[:])
            r1_t = cp.tile([H, 6 * H], dt.bfloat16)
            nc.sync.dma_start(out=r1_t[:], in_=R1[:])
            bias1_t = cp.tile([1, 6 * H], dt.bfloat16)
            nc.sync.dma_start(out=bias1_t[:], in_=bias1[:])
            r2_t = cp.tile([H, 4 * H], dt.bfloat16)
            nc.sync.dma_start(out=r2_t[:], in_=R2[:])
            bias2_t = cp.tile([1, 4 * H], dt.bfloat16)
            nc.sync.dma_start(out=bias2_t[:], in_=bias2[:])
            wfc_t = cp.tile([H, C_OUT], dt.bfloat16)
            nc.sync.dma_start(out=wfc_t[:], in_=WfcT[:])
            bfc_t = cp.tile([1, C_OUT], dt.bfloat16)
            nc.sync.dma_start(out=bfc_t[:], in_=bfc[:])
            b1_t = cp.tile([1, H], dt.bfloat16)
            nc.sync.dma_start(out=b1_t[:], in_=b1[:])
            invc_t = cp.tile([128, G], dt.float32)
            nc.sync.dma_start(out=invc_t[:], in_=invc[:])
            gcnt_t = cp.tile([1, meta["n_items"]], dt.int32)
            nc.sync.dma_start(out=gcnt_t[:], in_=gcntd[:])
            item_base = np.zeros(G + 1, np.int64)
            for g in range(G):
                item_base[g + 1] = item_base[g] + len(sched[g])

            UB = bool(meta.get("use_bias", True))
            # k SBUF-resident (written P3, read P6)
            kres = cp.tile([128, G, H], dt.bfloat16)

            with tc.tile_pool(name="resa", bufs=1) as ra:

                # ------------ P1: dense layer 1 (local nodes) -------------
                with (
                    tc.tile_pool(name="p1sb", bufs=3) as sb,
                    tc.tile_pool(name="p1ps", bufs=2, space="PSUM") as ps,
                    tc.tile_pool(name="p1ps2", bufs=2, space="PSUM") as ps2,
                ):
                    NB = 512
                    for s0 in range(0, NP, NB):
                        n = min(NB, NP - s0)
                        xt_t = sb.tile([IN_DIM, n], dt.bfloat16, tag="xt")
                        nc.sync.dma_start(out=xt_t[:], in_=xT[:, s0:s0 + n])
                        p1 = ps.tile([128, NB], dt.float32, tag="p1")
                        nc.tensor.matmul(p1[:, :n], lhsT=w1t_t[:],
                                         rhs=xt_t[:], start=True, stop=not UB)
                        if UB:
                            nc.tensor.matmul(p1[:, :n], lhsT=b1_t[:],
                                             rhs=ones_col[:, :n],
                                             start=False, stop=True)
                        x1t = sb.tile([128, NB], dt.bfloat16, tag="x1t")
                        nc.scalar.activation(
                            x1t[:, :n], p1[:, :n],
                            mybir.ActivationFunctionType.Relu)
                        for nb in range(0, n, 128):
                            m = min(128, n - nb)
                            g = (s0 + nb) // 128
                            p2 = ps2.tile([128, 6 * H], dt.float32, tag="p2")
                            lhsT = x1t[:, nb:nb + m]
                            nc.tensor.matmul(p2[:m, :512], lhsT=lhsT,
                                             rhs=r1_t[:, :512],
                                             start=True, stop=not UB)
                            if UB:
                                nc.tensor.matmul(p2[:m, :512],
                                                 lhsT=ones_col[:, :m],
                                                 rhs=bias1_t[:, :512],
                                                 start=False, stop=True)
                            nc.tensor.matmul(p2[:m, 512:], lhsT=lhsT,
                                             rhs=r1_t[:, 512:],
                                             start=True, stop=not UB)
                            if UB:
                                nc.tensor.matmul(p2[:m, 512:],
                                                 lhsT=ones_col[:, :m],
                                                 rhs=bias1_t[:, 512:],
                                                 start=False, stop=True)
                            # xl -> DRAM (AllGather input); gamma-beta
                            # -> resident SBUF
                            xlt = sb.tile([128, H], dt.bfloat16, tag="xlt")
                            nc.scalar.copy(xlt[:m, :], p2[:m, :H])
                            nc.sync.dma_start(
                                out=xl_loc[s0 + nb:s0 + nb + m, :],
                                in_=xlt[:m, :])
                            gbt = sb.tile([128, 2 * H], dt.bfloat16,
                                          tag="gbt")
                            nc.scalar.copy(gbt[:m, :], p2[:m, H:3 * H])
                            nc.sync.dma_start(
                                out=gb_loc[s0 + nb:s0 + nb + m, :],
                                in_=gbt[:m, :])
                            # FiLM skip: relu(gs * xls + bs) -> resident
                            sks = sb.tile([128, 3 * H], dt.bfloat16,
                                          tag="sks")
                            nc.scalar.copy(sks[:m, :], p2[:m, 384:768])
                            tmp = sb.tile([128, H], dt.bfloat16, tag="tmp")
                            nc.vector.tensor_mul(tmp[:m, :], sks[:m, :H],
                                                 sks[:m, 2 * H:])
                            pre = sb.tile([128, H], dt.bfloat16, tag="pre")
                            nc.vector.tensor_add(pre[:m, :], tmp[:m, :],
                                                 sks[:m, H:2 * H])
                            sk = sb.tile([128, H], dt.bfloat16, tag="sk")
                            nc.vector.tensor_scalar_max(sk[:m, :],
                                                        pre[:m, :], 0.0)
                            nc.sync.dma_start(
                                out=skip_loc[s0 + nb:s0 + nb + m, :],
                                in_=sk[:m, :])

                # ------------ P2: AllGather xl ----------------------------
                nc.gpsimd.collective_compute(
                    "AllGather", mybir.AluOpType.bypass,
                    replica_groups=[list(range(N_CORES))],
                    ins=[xl_loc[:]], outs=[xl_full[:]],
                )

                # ------------ P3: FiLM edge phase + dense layer 2 ---------
                with (
                    tc.tile_pool(name="p3sb", bufs=3) as sb,
                    tc.tile_pool(name="p3g", bufs=3) as gpool,
                    tc.tile_pool(name="p3ps", bufs=2, space="PSUM") as pse,
                    tc.tile_pool(name="p3pa", bufs=1, space="PSUM") as psa,
                    tc.tile_pool(name="p3pt", bufs=1, space="PSUM") as pst,
                    tc.tile_pool(name="p3pd", bufs=2, space="PSUM") as psd,
                ):
                    prev3 = None

                    def dense2(g, x2):
                        pt = pst.tile([128, H], dt.bfloat16, tag="pt")
                        nc.tensor.transpose(pt[:], x2[:], ident[:])
                        x2t = sb.tile([128, H], dt.bfloat16, tag="x2t")
                        nc.scalar.copy(x2t[:], pt[:])
                        p2 = psd.tile([128, 4 * H], dt.float32, tag="p2d")
                        nc.tensor.matmul(p2[:], lhsT=x2t[:], rhs=r2_t[:],
                                         start=True, stop=not UB)
                        if UB:
                            nc.tensor.matmul(p2[:], lhsT=ones_col[:, :128],
                                             rhs=bias2_t[:], start=False,
                                             stop=True)
                        nc.scalar.copy(kres[:, g, :], p2[:, :H])
                        s2t = sb.tile([128, H], dt.bfloat16, tag="s2t")
                        nc.scalar.copy(s2t[:], p2[:, H:2 * H])
                        nc.sync.dma_start(
                            out=skip2_loc[g * 128:(g + 1) * 128, :],
                            in_=s2t[:])
                        qv = sb.tile([128, 2 * H], dt.bfloat16, tag="qv")
                        nc.scalar.copy(qv[:], p2[:, 2 * H:])
                        nc.sync.dma_start(
                            out=qv_loc[g * 128:(g + 1) * 128, :], in_=qv[:])

                    for g in range(G):
                        C = int(Cg[g])
                        co = int(g_coff[g])
                        blob_g = sb.tile([128, CMAX * 272], dt.uint8,
                                         tag="blob")
                        nc.sync.dma_start(
                            out=blob_g[:, :C * 272],
                            in_=blobd[:, co * 272:(co + C) * 272])
                        idx_g = blob_g[:, :C * 16].bitcast(dt.int16)
                        gt_g = blob_g[:, C * 16:C * 144].bitcast(dt.float8e4)
                        gtt_g = blob_g[:, C * 144:C * 272].bitcast(
                            dt.float8e4)
                        gbg_t = sb.tile([128, 2 * H], dt.bfloat16,
                                        tag="gbg")
                        nc.sync.dma_start(
                            out=gbg_t[:],
                            in_=gb_loc[g * 128:(g + 1) * 128, :])
                        gb_g = gbg_t[:]
                        skp_g = sb.tile([128, H], dt.bfloat16, tag="skp")
                        nc.sync.dma_start(
                            out=skp_g[:],
                            in_=skip_loc[g * 128:(g + 1) * 128, :])
                        # gathers (-1 idx pads are skipped by SWDGE)
                        xg = gpool.tile([128, CMAX, H], dt.bfloat16,
                                        tag="xg")
                        if g < 2:
                            nc.vector.memset(xg[:], 0.0)
                        for qi, (b, nidx, coloff, _, _, _) in enumerate(
                                sched[g]):
                            nrow = min(BANK, NFULL - b * BANK)
                            it = int(item_base[g]) + qi
                            with nc.gpsimd.register() as reg:
                                nc.gpsimd.reg_load(reg,
                                                   gcnt_t[0:1, it:it + 1])
                                nc.gpsimd.dma_gather(
                                    xg[:, coloff:coloff + nidx // 128, :],
                                    xl_full[b * BANK:b * BANK + nrow, :],
                                    idx_g[:, coloff * 8:
                                          coloff * 8 + nidx // 16],
                                    nidx, reg, H, queue_num=qi % 4,
                                )
                        # gamma/beta expansion + msg, batched; reduce
                        # matmuls trail one batch behind so the PE never
                        # stalls on the whole group's DVE work.
                        msg = gpool.tile([128, CMAX, H], dt.bfloat16,
                                         tag="msg")
                        pa = psa.tile([128, H], dt.float32, tag="pa")
                        nbat = (C + 3) // 4

                        def red3(bb, C=C, gt_g=gt_g, pa=pa, msg=msg):
                            for j in range(min(4, C - 4 * bb)):
                                c = 4 * bb + j
                                nc.tensor.matmul(
                                    pa[:],
                                    lhsT=gt_g[:, c * 128:(c + 1) * 128],
                                    rhs=msg[:, c, :],
                                    start=(c == 0), stop=(c == C - 1))

                        for bb in range(nbat):
                            c0 = 4 * bb
                            cn = min(4, C - c0)
                            pe = pse.tile([128, 4, 2 * H], dt.float32,
                                          tag="pe")
                            for j in range(cn):
                                c = c0 + j
                                nc.tensor.matmul(
                                    pe[:, j, :],
                                    lhsT=gtt_g[:, c * 128:(c + 1) * 128],
                                    rhs=gb_g, start=True, stop=True)
                            m0 = gpool.tile([128, 4, H], dt.bfloat16,
                                            tag="m0")
                            nc.vector.tensor_mul(m0[:, :cn, :],
                                                 pe[:, :cn, :H],
                                                 xg[:, c0:c0 + cn, :])
                            m1 = gpool.tile([128, 4, H], dt.bfloat16,
                                            tag="m1")
                            nc.vector.tensor_add(m1[:, :cn, :],
                                                 m0[:, :cn, :],
                                                 pe[:, :cn, H:])
                            nc.scalar.activation(
                                msg[:, c0:c0 + cn, :], m1[:, :cn, :],
                                mybir.ActivationFunctionType.Relu)
                            if bb >= 1:
                                red3(bb - 1)
                        red3(nbat - 1)
                        # x2 = relu(skip + agg/cnt)
                        aggm = sb.tile([128, H], dt.float32, tag="aggm")
                        nc.vector.tensor_scalar_mul(aggm[:], pa[:],
                                                    invc_t[:, g:g + 1])
                        x2p = sb.tile([128, H], dt.float32, tag="x2p")
                        nc.vector.tensor_add(x2p[:], aggm[:], skp_g[:])
                        x2 = sb.tile([128, H], dt.bfloat16, tag="x2")
                        nc.vector.tensor_scalar_max(x2[:], x2p[:], 0.0)
                        # dense layer 2, staggered one group behind so the
                        # in-order PE never waits on this group's x2 chain
                        if prev3 is not None:
                            dense2(*prev3)
                        prev3 = (g, x2)
                    dense2(*prev3)

            # ---------------- P5: AllGather qv ----------------------------
            nc.gpsimd.collective_compute(
                "AllGather", mybir.AluOpType.bypass,
                replica_groups=[list(range(N_CORES))],
                ins=[qv_loc[:]], outs=[qv_full[:]],
            )

            # ---------------- P6: ResGated edge phase ---------------------
            with (
                tc.tile_pool(name="p6sb", bufs=2) as sb,
                tc.tile_pool(name="p6g", bufs=3) as gpool,
                tc.tile_pool(name="p6ps", bufs=2, space="PSUM") as psk,
                tc.tile_pool(name="p6pa", bufs=1, space="PSUM") as psa,
                tc.tile_pool(name="p6pt", bufs=1, space="PSUM") as pst,
                tc.tile_pool(name="p6pl", bufs=1, space="PSUM") as psl,
            ):
                prev6 = None

                def fcout(g, x3):
                    pt = pst.tile([128, H], dt.bfloat16, tag="pt6")
                    nc.tensor.transpose(pt[:], x3[:], ident[:])
                    x3t = sb.tile([128, H], dt.bfloat16, tag="x3t")
                    nc.scalar.copy(x3t[:], pt[:])
                    # logits_T = Wfc @ x3T + bfc
                    pl = psl.tile([C_OUT, 128], dt.float32, tag="pl")
                    nc.tensor.matmul(pl[:], lhsT=wfc_t[:], rhs=x3t[:],
                                     start=True, stop=not UB)
                    if UB:
                        nc.tensor.matmul(pl[:], lhsT=bfc_t[:],
                                         rhs=ones_col[:, :128],
                                         start=False, stop=True)
                    lt = sb.tile([C_OUT, 128], dt.float32, tag="lt")
                    nc.scalar.copy(lt[:], pl[:])
                    nc.sync.dma_start(out=outT[:, g * 128:(g + 1) * 128],
                                      in_=lt[:])

                for g in range(G):
                    C = int(Cg[g])
                    co = int(g_coff[g])
                    blob_g = sb.tile([128, CMAX * 272], dt.uint8, tag="blob6")
                    nc.sync.dma_start(out=blob_g[:, :C * 272],
                                      in_=blobd[:, co * 272:(co + C) * 272])
                    idx_g = blob_g[:, :C * 16].bitcast(dt.int16)
                    gt_g = blob_g[:, C * 16:C * 144].bitcast(dt.float8e4)
                    gtt_g = blob_g[:, C * 144:C * 272].bitcast(dt.float8e4)
                    sk2_g = sb.tile([128, H], dt.bfloat16, tag="sk2")
                    nc.sync.dma_start(
                        out=sk2_g[:],
                        in_=skip2_loc[g * 128:(g + 1) * 128, :])
                    qvg = gpool.tile([128, CMAX, 2 * H], dt.bfloat16,
                                     tag="qvg")
                    if g < 2:
                        nc.vector.memset(qvg[:], 0.0)
                    for qi, (b, nidx, coloff, _, _, _) in enumerate(sched[g]):
                        nrow = min(BANK, NFULL - b * BANK)
                        it = int(item_base[g]) + qi
                        with nc.gpsimd.register() as reg:
                            nc.gpsimd.reg_load(reg, gcnt_t[0:1, it:it + 1])
                            nc.gpsimd.dma_gather(
                                qvg[:, coloff:coloff + nidx // 128, :],
                                qv_full[b * BANK:b * BANK + nrow, :],
                                idx_g[:, coloff * 8:coloff * 8 + nidx // 16],
                                nidx, reg, 2 * H, queue_num=qi % 4,
                            )
                    # k-expansion + eta + contrib, batched; reduce trails
                    # one batch behind.
                    ctb = gpool.tile([128, CMAX, H], dt.bfloat16, tag="ctb")
                    pa = psa.tile([128, H], dt.float32, tag="pa6")
                    nbat = (C + 3) // 4

                    def red6(bb, C=C, gt_g=gt_g, pa=pa, ctb=ctb):
                        for j in range(min(4, C - 4 * bb)):
                            c = 4 * bb + j
                            nc.tensor.matmul(
                                pa[:],
                                lhsT=gt_g[:, c * 128:(c + 1) * 128],
                                rhs=ctb[:, c, :],
                                start=(c == 0), stop=(c == C - 1))

                    for bb in range(nbat):
                        c0 = 4 * bb
                        cn = min(4, C - c0)
                        pk = psk.tile([128, 4, H], dt.float32, tag="pk")
                        for j in range(cn):
                            c = c0 + j
                            nc.tensor.matmul(
                                pk[:, j, :],
                                lhsT=gtt_g[:, c * 128:(c + 1) * 128],
                                rhs=kres[:, g, :], start=True, stop=True)
                        kq = gpool.tile([128, 4, H], dt.bfloat16, tag="kq")
                        nc.vector.tensor_add(kq[:, :cn, :], pk[:, :cn, :],
                                             qvg[:, c0:c0 + cn, :H])
                        eta = gpool.tile([128, 4, H], dt.bfloat16, tag="eta")
                        nc.scalar.activation(
                            eta[:, :cn, :], kq[:, :cn, :],
                            mybir.ActivationFunctionType.Sigmoid)
                        nc.vector.tensor_mul(ctb[:, c0:c0 + cn, :],
                                             eta[:, :cn, :],
                                             qvg[:, c0:c0 + cn, H:])
                        if bb >= 1:
                            red6(bb - 1)
                    red6(nbat - 1)
                    # x3 = relu(s2 + skip2)
                    x3p = sb.tile([128, H], dt.float32, tag="x3p")
                    nc.vector.tensor_add(x3p[:], pa[:], sk2_g[:])
                    x3 = sb.tile([128, H], dt.bfloat16, tag="x3")
                    nc.vector.tensor_scalar_max(x3[:], x3p[:], 0.0)
                    if prev6 is not None:
                        fcout(*prev6)
                    prev6 = (g, x3)
                fcout(*prev6)

    nc.compile()
    return nc


# ---------------------------------------------------------------------------
# Runner (PJRT shard_map, compile once)
# ---------------------------------------------------------------------------

class _Runner:
    def __init__(self, nc):
        import jax
        import concourse.mybir as mybir
        from concourse import bass2jax
        from concourse.bass2jax import _bass_exec_p, install_neuronx_cc_hook
        from jax.sharding import Mesh, PartitionSpec
        try:
            from jax.experimental.shard_map import shard_map
        except ImportError:
            from jax.sharding import shard_map
        install_neuronx_cc_hook()
        self.jax = jax
        partition_name = (nc.partition_id_tensor.name
                          if nc.partition_id_tensor else None)
        in_names, out_names, out_avals, zero_outs = [], [], [], []
        for alloc in nc.m.functions[0].allocations:
            if not isinstance(alloc, mybir.MemoryLocationSet):
                continue
            name = alloc.memorylocations[0].name
            if alloc.kind == "ExternalInput":
                if name != partition_name:
                    in_names.append(name)
            elif alloc.kind == "ExternalOutput":
                out_names.append(name)
                shape = tuple(alloc.tensor_shape)
                dtype = mybir.dt.np(alloc.dtype)
                out_avals.append(jax.core.ShapedArray(shape, dtype))
                zero_outs.append(np.zeros(shape, dtype))
        self.in_names, self.out_names = in_names, out_names
        self.out_avals, self.zero_outs = out_avals, zero_outs
        n_params, n_outs = len(in_names), len(out_avals)
        all_in = list(in_names) + list(out_names)
        if partition_name is not None:
            all_in.append(partition_name)

        def _body(*args):
            operands = list(args)
            if partition_name is not None:
                operands.append(bass2jax.partition_id_tensor())
            return tuple(_bass_exec_p.bind(
                *operands, out_avals=tuple(out_avals),
                in_names=tuple(all_in), out_names=tuple(out_names),
                lowering_input_output_aliases=(),
                sim_require_finite=True, sim_require_nnan=True, nc=nc))

        devices = jax.devices()[:N_CORES]
        self.mesh = Mesh(np.asarray(devices), ("core",))
        specs_in = (PartitionSpec("core"),) * (n_params + n_outs)
        specs_out = (PartitionSpec("core"),) * len(out_names)
        self.fn = jax.jit(
            shard_map(_body, mesh=self.mesh, in_specs=specs_in,
                      out_specs=specs_out, check_rep=False),
            keep_unused=True)

    def run(self, in_maps):
        jax = self.jax
        from jax.sharding import NamedSharding, PartitionSpec
        per_core = [[np.asarray(m[n]) for n in self.in_names]
                    for m in in_maps]
        concat = [np.concatenate([per_core[c][i] for c in range(N_CORES)], 0)
                  for i in range(len(self.in_names))]
        zeros = [np.zeros((N_CORES * z.shape[0], *z.shape[1:]), z.dtype)
                 for z in self.zero_outs]
        sh = NamedSharding(self.mesh, PartitionSpec("core"))
        args = [jax.device_put(a, sh) for a in concat + zeros]
        outs = self.fn(*args)
        jax.block_until_ready(outs)
        return [
            {n: np.asarray(outs[i]).reshape(N_CORES,
                                            *self.out_avals[i].shape)[c]
             for i, n in enumerate(self.out_names)}
            for c in range(N_CORES)
        ], (args, outs)


_CACHE = {}


def kernel(**inputs) -> np.ndarray:
    edge_index = np.asarray(inputs["edge_index"])
    x = np.asarray(inputs["x"], dtype=np.float32)

    meta, idx_blobs, gt_blobs, gtt_blobs, inv_cnts, gcnts = _prep_edges(
        edge_index)
    meta["use_bias"] = any(
        float(np.abs(np.asarray(inputs[n], np.float32)).max()) > 0
        for n in ("b1", "bf", "bfs", "bk", "bq", "bv", "bres", "bfc"))
    w = _prep_weights(inputs)

    key = "k"
    if key not in _CACHE:
        nc = _build(meta)
        _CACHE[key] = (_Runner(nc), meta)
    runner, _ = _CACHE[key]

    in_maps = []
    for c in range(N_CORES):
        xT_c = np.zeros((IN_DIM, NP), BF16)
        xT_c[:, :NC] = x[c * NC:(c + 1) * NC, :].T.astype(BF16)
        in_maps.append({
            "xT": xT_c, "W1T": w["W1T"], "R1": w["R1"], "bias1": w["bias1"],
            "R2": w["R2"], "bias2": w["bias2"], "WfcT": w["WfcT"],
            "bfc": w["bfc"], "b1": w["b1"],
            "blobd": idx_blobs[c], "invc": inv_cnts[c], "gcnt": gcnts[c],
        })
    results, _ = runner.run(in_maps)
    logits = np.concatenate(
        [results[c]["outT"][:, :NC].T for c in range(N_CORES)], axis=0
    ).astype(np.float32)
    return (logits, logits)

